# revision 1
# baseline (speedup 1.0000x reference)
"""4-layer tanh RNN on 8 Trainium2 NeuronCores.

Strategy: 4-stage layer pipeline x 2-way batch split. Core c handles
layer c//2 for batch half c%2. Time is processed in blocks of T=32 steps;
each round every core: gathers its input block (previous stage's output)
from the round's AllGather, projects it (x @ WxT + b), runs 32 recurrence
steps (weight-stationary bf16 matmuls, zT[d_out,b] layout so h never needs
a transpose), then contributes its output block to the next AllGather.
Cross-core addressing is SPMD-uniform: per-core *data* (indirect-DMA gather
indices, carry/init masks, zeroed feeds) encodes each core's role.

Compute dtype bf16 (PE fp32 is 4x slower), fp32 PSUM accumulation, fp32
tanh. Measured end-to-end absmax-relative error vs the fp32 reference
~1e-2.
"""
import sys
import numpy as np

if "/opt/trn_rl_repo" not in sys.path:
    sys.path.insert(0, "/opt/trn_rl_repo")

import ml_dtypes

BF = ml_dtypes.bfloat16

# Problem config (hardcoded per contract)
import os as _os
B, L, D, NL = 16, 512, 1024, 4
if _os.environ.get("RNN_SMALL"):  # dev-only fast config; L shrinks
    L = int(_os.environ["RNN_SMALL"])
P = 128
KT = D // P          # 8 k-tiles (contraction)
MT = D // P          # 8 m-tiles (output)
BC = B // 2          # 8 = per-core batch half
T = 32               # timesteps per block
NB = L // T          # 16 blocks
ROUNDS = NB + NL - 1  # 19
N_CORES = 8
BLK_COLS = MT * T * BC  # 2048 block columns: col = m*T*BC + t*BC + b

_cache = {}


def _build():
    import concourse.bass as bass
    import concourse.mybir as mybir
    import concourse.tile as tile
    from concourse import bacc
    from concourse.tile import add_dep_helper

    F32 = mybir.dt.float32
    BF16 = mybir.dt.bfloat16
    I32 = mybir.dt.int32
    Tanh = mybir.ActivationFunctionType.Tanh

    nc = bacc.Bacc("TRN2", target_bir_lowering=False, debug=False,
                   num_devices=N_CORES)

    # ---- I/O ----
    whT = nc.dram_tensor("whT", [P, KT * MT * P], BF16, kind="ExternalInput")
    wxT = nc.dram_tensor("wxT", [P, KT * MT * P], BF16, kind="ExternalInput")
    bias = nc.dram_tensor("bias", [P, MT], F32, kind="ExternalInput")
    carry = nc.dram_tensor("carry", [ROUNDS, P, KT * BC], mybir.dt.uint8, kind="ExternalInput")
    cinit = nc.dram_tensor("cinit", [ROUNDS, P, KT * BC], BF16, kind="ExternalInput")
    gidx0 = nc.dram_tensor("gidx0", [P, 1], I32, kind="ExternalInput")
    gidx = nc.dram_tensor("gidx", [P, 1], I32, kind="ExternalInput")
    x0t = nc.dram_tensor("x0t", [ROUNDS, P, BLK_COLS], BF16, kind="ExternalInput")
    ag_init = nc.dram_tensor("ag_init", [2 * P, BLK_COLS], BF16, kind="ExternalInput")
    out = nc.dram_tensor("out", [ROUNDS, P, BLK_COLS], F32, kind="ExternalOutput")

    debug = bool(_os.environ.get("RNN_DEBUG"))
    if debug:
        dbg_xb = nc.dram_tensor("dbg_xb", [ROUNDS, P, BLK_COLS], F32,
                                kind="ExternalOutput")
        dbg_xw = nc.dram_tensor("dbg_xw", [ROUNDS, P, BLK_COLS], F32,
                                kind="ExternalOutput")

    ag_ins = [nc.dram_tensor(f"ag_in_{r}", [2 * P, BLK_COLS], BF16)
              for r in range(ROUNDS - 1)]
    ag_outs = [nc.dram_tensor(f"ag_out_{r}", [N_CORES * 2 * P, BLK_COLS], BF16,
                              addr_space="Shared")
               for r in range(ROUNDS - 1)]

    with tile.TileContext(nc) as tc:
        with (
            tc.tile_pool(name="const", bufs=1) as cpool,
            tc.tile_pool(name="xblk", bufs=2) as xpool,
            tc.tile_pool(name="xw", bufs=1) as xwpool,
            tc.tile_pool(name="blk", bufs=1) as blkpool,
            tc.tile_pool(name="hs", bufs=2) as hspool,
            tc.tile_pool(name="o32", bufs=2) as opool,
            tc.tile_pool(name="z", bufs=4) as zpool,
            tc.tile_pool(name="psr", bufs=4, space="PSUM") as prpool,
            tc.tile_pool(name="psp", bufs=2, space="PSUM") as pppool,
        ):
            wh_sb = cpool.tile([P, KT, MT, P], BF16, tag="wh")
            nc.sync.dma_start(wh_sb[:], whT.ap().rearrange("p (k m q) -> p k m q", k=KT, m=MT))
            wx_sb = cpool.tile([P, KT, MT, P], BF16, tag="wx")
            nc.sync.dma_start(wx_sb[:], wxT.ap().rearrange("p (k m q) -> p k m q", k=KT, m=MT))
            bias_sb = cpool.tile([P, MT], F32, tag="bias")
            nc.sync.dma_start(bias_sb[:], bias[:])
            carry_sb = cpool.tile([P, ROUNDS, KT * BC], mybir.dt.uint8, tag="carry")
            nc.sync.dma_start(carry_sb[:], carry.ap().rearrange("r p c -> p r c"))
            cinit_sb = cpool.tile([P, ROUNDS, KT * BC], BF16, tag="cinit")
            nc.sync.dma_start(cinit_sb[:], cinit.ap().rearrange("r p c -> p r c"))
            gidx0_sb = cpool.tile([P, 1], I32, tag="gidx0")
            nc.sync.dma_start(gidx0_sb[:], gidx0[:])
            gidx_sb = cpool.tile([P, 1], I32, tag="gidx")
            nc.sync.dma_start(gidx_sb[:], gidx[:])

            # two persistent block buffers, alternated by round parity
            blkA = blkpool.tile([P, MT, T, BC], BF16, tag="blkA")
            blkB = blkpool.tile([P, MT, T, BC], BF16, tag="blkB")
            nc.vector.memset(blkA[:], 0.0)
            nc.vector.memset(blkB[:], 0.0)

            xw_sb = xwpool.tile([P, MT, T, BC], F32, tag="xw")

            cc_prev = None
            for r in range(ROUNDS):
                cur = blkA if r % 2 == 0 else blkB
                prev = blkB if r % 2 == 0 else blkA

                # ---- 1. gather input block from previous round's AG ----
                src = ag_init if r == 0 else ag_outs[r - 1]
                idx = gidx0_sb if r == 0 else gidx_sb
                xblk = xpool.tile([P, KT * T * BC], BF16, tag="xblk")
                g = nc.gpsimd.indirect_dma_start(
                    out=xblk[:],
                    out_offset=None,
                    in_=src[:],
                    in_offset=bass.IndirectOffsetOnAxis(ap=idx[:, :1], axis=0),
                )
                if cc_prev is not None:
                    add_dep_helper(g.ins, cc_prev.ins, sync=True, reason="gather after AG")

                # ---- 2. projection: xw[m] = sum_k WxT(k,m).T @ xblk[k] + bias[m] ----
                for m in range(MT):
                    pp = pppool.tile([P, T, BC], mybir.dt.float32, tag="pp")
                    for k in range(KT):
                        nc.tensor.matmul(
                            pp[:],
                            wx_sb[:, k, m, :],
                            xblk[:, k * T * BC:(k + 1) * T * BC],
                            start=(k == 0),
                            stop=(k == KT - 1),
                        )
                    nc.vector.tensor_tensor(
                        out=xw_sb[:, m],
                        in0=pp[:],
                        in1=bias_sb[:, m, None].to_broadcast((P, T, BC)),
                        op=mybir.AluOpType.add,
                    )

                if debug:
                    dxb = opool.tile([P, BLK_COLS], F32, tag="dxb")
                    nc.vector.tensor_copy(dxb[:], xblk[:])
                    nc.sync.dma_start(dbg_xb[r], dxb[:])
                    dxw = opool.tile([P, BLK_COLS], F32, tag="dxw")
                    nc.vector.tensor_copy(dxw[:], xw_sb[:])
                    nc.sync.dma_start(dbg_xw[r], dxw[:])

                # ---- 3. h_start = carry ? prev_block_tail : cinit ----
                hstart = hspool.tile([P, KT * BC], BF16, tag="hs")
                nc.vector.tensor_copy(hstart[:], cinit_sb[:, r])
                nc.vector.copy_predicated(
                    hstart[:], carry_sb[:, r], prev[:, :, T - 1, :]
                )

                # ---- 4. recurrence over T steps ----
                for t in range(T):
                    for half in range(2):
                        ps = prpool.tile([P, 4, BC], mybir.dt.float32, tag="ps")
                        # One accumulation group per PSUM bank: start=True only
                        # on the very first matmul (it clears has_written for
                        # the WHOLE bank); later regions overwrite-on-clear
                        # then accumulate. k-outer so the clear runs first.
                        first_mm = None
                        for k in range(KT):
                            if t == 0:
                                rhs = hstart[:, k * BC:(k + 1) * BC]
                            else:
                                rhs = cur[:, k, t - 1, :]
                            for mi in range(4):
                                m = half * 4 + mi
                                mm = nc.tensor.matmul(
                                    ps[:, mi, :],
                                    wh_sb[:, k, m, :],
                                    rhs,
                                    start=(k == 0 and mi == 0),
                                    stop=(k == KT - 1 and mi == 3),
                                    skip_group_check=True,
                                )
                                if first_mm is None:
                                    first_mm = mm
                                elif k == 0:
                                    add_dep_helper(mm.ins, first_mm.ins, sync=False,
                                                   reason="bank clear first")
                        z = zpool.tile([P, 4, BC], mybir.dt.float32, tag="z")
                        nc.vector.tensor_tensor(
                            out=z[:],
                            in0=ps[:],
                            in1=xw_sb[:, half * 4:(half + 1) * 4, t, :],
                            op=mybir.AluOpType.add,
                        )
                        nc.scalar.activation(
                            cur[:, half * 4:(half + 1) * 4, t, :], z[:], Tanh
                        )

                # ---- 5. write fp32 output block ----
                o32 = opool.tile([P, MT * T * BC], F32, tag="o32")
                nc.vector.tensor_copy(o32[:], cur[:])
                nc.sync.dma_start(out[r], o32[:])

                # ---- 6. contribute to AG (block + x-feed) and trigger ----
                if r < ROUNDS - 1:
                    d1 = nc.sync.dma_start(
                        ag_ins[r][0:P, :],
                        cur[:].rearrange("p m t b -> p (m t b)"),
                    )
                    d2 = nc.sync.dma_start(ag_ins[r][P:2 * P, :], x0t[r + 1])
                    cc = nc.gpsimd.collective_compute(
                        "AllGather",
                        mybir.AluOpType.bypass,
                        replica_groups=[list(range(N_CORES))],
                        ins=[ag_ins[r][:]],
                        outs=[ag_outs[r][:]],
                    )
                    add_dep_helper(cc.ins, d1.ins, sync=True, reason="AG after blk dma")
                    add_dep_helper(cc.ins, d2.ins, sync=True, reason="AG after feed dma")
                    cc_prev = cc
    nc.compile()
    return nc


def _prep_inputs(X, h0s, W, b):
    """Build the 8 per-core input maps."""
    in_maps = []
    for c in range(N_CORES):
        s, j = c // 2, c % 2
        Wl = np.asarray(W[s], dtype=np.float32)
        Wx, Wh = Wl[:, :D], Wl[:, D:]

        def tiles(M):  # M: [e, d] -> lhsT tiles [p, (k, m, q)]
            A = M.reshape(MT, P, KT, P)          # [m, q, k, p]
            return np.ascontiguousarray(
                A.transpose(3, 2, 0, 1).reshape(P, KT * MT * P)).astype(BF)

        whT = tiles(Wh)
        wxT = tiles(Wx)
        bias = np.ascontiguousarray(
            np.asarray(b[s], np.float32).reshape(MT, P).T)

        hin = np.asarray(h0s[s, BC * j:BC * (j + 1)], np.float32)  # [b, d]
        hinit = np.ascontiguousarray(
            hin.reshape(BC, KT, P).transpose(2, 1, 0).reshape(P, KT * BC)).astype(BF)

        carry = np.zeros((ROUNDS, P, KT * BC), np.uint8)
        cinit = np.zeros((ROUNDS, P, KT * BC), BF)
        for r in range(ROUNDS):
            if r > s:
                carry[r] = 1
            else:
                cinit[r] = hinit

        x0t = np.zeros((ROUNDS, P, BLK_COLS), BF)
        ag_init = np.zeros((2 * P, BLK_COLS), BF)
        if s == 0:
            Xj = np.asarray(X[BC * j:BC * (j + 1)], np.float32)  # [b, L, d]
            # [b, q, t, k, p] -> [q, p, k, t, b]
            Xb = Xj.reshape(BC, NB, T, KT, P).transpose(1, 4, 3, 2, 0)
            Xb = np.ascontiguousarray(Xb.reshape(NB, P, BLK_COLS)).astype(BF)
            x0t[1:NB] = Xb[1:]
            # block 0 goes into ag_init's feed half
            ag_init[P:2 * P, :] = Xb[0]
            gidx0 = (P + np.arange(P, dtype=np.int32)).reshape(P, 1)
            gidx = (c * 2 * P + P + np.arange(P, dtype=np.int32)).reshape(P, 1)
        else:
            gidx0 = np.arange(P, dtype=np.int32).reshape(P, 1)
            gidx = ((c - 2) * 2 * P + np.arange(P, dtype=np.int32)).reshape(P, 1)

        in_maps.append({
            "whT": whT, "wxT": wxT, "bias": bias,
            "carry": carry, "cinit": cinit,
            "gidx0": gidx0, "gidx": gidx,
            "x0t": x0t, "ag_init": ag_init,
        })
    return in_maps


def _extract(results):
    """Assemble full output [B, L, D] from stage-3 cores (6, 7)."""
    Y = np.empty((B, L, D), np.float32)
    for j in range(2):
        o = results[6 + j]["out"][NL - 1:NL - 1 + NB]   # [q, p, cols]
        o = o.reshape(NB, P, MT, T, BC).transpose(4, 0, 3, 2, 1)  # [b,q,t,m,p]
        Y[BC * j:BC * (j + 1)] = o.reshape(BC, L, D)
    return Y


def kernel(X, h0s, W, b, _trace=False):
    from concourse.bass_utils import run_bass_kernel_spmd

    if "nc" not in _cache:
        _cache["nc"] = _build()
    nc = _cache["nc"]
    in_maps = _prep_inputs(np.asarray(X), np.asarray(h0s), np.asarray(W),
                           np.asarray(b))
    res = run_bass_kernel_spmd(nc, in_maps, core_ids=list(range(N_CORES)),
                               trace=_trace)
    _cache["last_results"] = res
    return _extract(res.results)



# revision 9
# speedup vs baseline: 1.1482x; 1.1482x over previous
"""4-layer tanh RNN on 8 Trainium2 NeuronCores.

Strategy: 4-stage layer pipeline x 2-way batch split. Core c handles
layer c//2 for batch half c%2. Time is processed in blocks of T=32 steps;
each round every core: gathers its input block (previous stage's output),
projects it (x @ WxT + b), runs 32 recurrence steps (weight-stationary
bf16 matmuls, zT[d_out,b] layout so h never needs a transpose), then
hands its output block to the next stage.

v2 over the original:
- Stage handoff via two pairwise AllGathers (0.5MB->1MB along chain
  edges) instead of one 8-way AllGather (1MB->8MB), writing disjoint
  regions of a combined per-round board tensor; the x-feed for stage-0
  cores rides a third region staged by local DMA. One SPMD-uniform
  indirect gather with a per-core constant row index reads the board.
- xw is accumulated into the step's PSUM group via an identity matmul,
  eliminating the per-step DVE add; tanh reads PSUM directly.
- One PSUM accumulation group per step (all 8 m-tiles, bank-padded
  tile so pool bufs never share a bank), tanh split in m-halves so the
  next step's k<4 matmuls only wait on the first half.
- Projection matmuls interleave m-pairs across two PSUM regions to
  avoid back-to-back same-region accumulation stalls; the bias add is
  fused with the fp32->bf16 cast on DVE.

Compute dtype bf16 (PE fp32 is 4x slower), fp32 PSUM accumulation, fp32
tanh. xw is stored bf16 (feeds the identity matmul).
"""
import sys
import numpy as np

if "/opt/trn_rl_repo" not in sys.path:
    sys.path.insert(0, "/opt/trn_rl_repo")

import ml_dtypes

BF = ml_dtypes.bfloat16

# Problem config (hardcoded per contract)
import os as _os
B, L, D, NL = 16, 512, 1024, 4
P = 128
KT = D // P          # 8 k-tiles (contraction)
MT = D // P          # 8 m-tiles (output)
BC = B // 2          # 8 = per-core batch half
T = 32               # timesteps per block
NB = L // T          # 16 blocks
ROUNDS = NB + NL - 1  # 19
N_CORES = 8
BLK_COLS = MT * T * BC  # 2048 block columns: col = m*T*BC + t*BC + b

# one chain per batch half: stage s core is group position s, so each
# receiver reads its predecessor's block at rows (s-1)*P of the AG out.
# (The runtime rejects NEFFs mixing two different replica-group sets,
# so the chain edges ride one 4-core AllGather: 2MB out/core vs 8MB
# for the 8-way gather.)
GROUPS = [[0, 2, 4, 6], [1, 3, 5, 7]]

_cache = {}


def _build():
    import concourse.bass as bass
    import concourse.mybir as mybir
    import concourse.tile as tile
    from concourse import bacc
    from concourse.tile import add_dep_helper

    F32 = mybir.dt.float32
    BF16 = mybir.dt.bfloat16
    I32 = mybir.dt.int32
    Tanh = mybir.ActivationFunctionType.Tanh

    nc = bacc.Bacc("TRN2", target_bir_lowering=False, debug=False,
                   num_devices=N_CORES)

    # ---- I/O ----
    whT = nc.dram_tensor("whT", [P, KT * MT * P], BF16, kind="ExternalInput")
    wxT = nc.dram_tensor("wxT", [P, KT * MT * P], BF16, kind="ExternalInput")
    bias = nc.dram_tensor("bias", [P, MT], F32, kind="ExternalInput")
    carry = nc.dram_tensor("carry", [ROUNDS, P, KT * BC], mybir.dt.uint8, kind="ExternalInput")
    cinit = nc.dram_tensor("cinit", [ROUNDS, P, KT * BC], BF16, kind="ExternalInput")
    gidx = nc.dram_tensor("gidx", [P, 1], I32, kind="ExternalInput")
    ident = nc.dram_tensor("ident", [P, P], BF16, kind="ExternalInput")
    x0t = nc.dram_tensor("x0t", [ROUNDS, P, BLK_COLS], BF16, kind="ExternalInput")
    out = nc.dram_tensor("out", [ROUNDS, P, BLK_COLS], F32, kind="ExternalOutput")

    # handoff board per round: rows [0:4P) AllGather out (one row-block
    # per chain stage), [4P:5P) x-feed (staged by local DMA from x0t).
    # Local (non-shared) outputs: 2-core groups don't support the shared
    # scratchpad, and AllGather lands each group's blocks in every
    # member's local copy anyway, which is all the gather reads.
    hands = [nc.dram_tensor(f"hand_{r}", [5 * P, BLK_COLS], BF16)
             for r in range(ROUNDS)]
    hins = [nc.dram_tensor(f"hin_{r}", [P, BLK_COLS], BF16)
            for r in range(ROUNDS - 1)]

    with tile.TileContext(nc) as tc:
        with (
            tc.tile_pool(name="const", bufs=1) as cpool,
            tc.tile_pool(name="xblk", bufs=2) as xpool,
            tc.tile_pool(name="xw", bufs=1) as xwpool,
            tc.tile_pool(name="blk", bufs=1) as blkpool,
            tc.tile_pool(name="hs", bufs=2) as hspool,
            tc.tile_pool(name="o32", bufs=2) as opool,
            tc.tile_pool(name="psr", bufs=2, space="PSUM") as prpool,
            tc.tile_pool(name="psp", bufs=4, space="PSUM") as pppool,
        ):
            wh_sb = cpool.tile([P, KT, MT, P], BF16, tag="wh")
            nc.sync.dma_start(wh_sb[:], whT.ap().rearrange("p (k m q) -> p k m q", k=KT, m=MT))
            wx_sb = cpool.tile([P, KT, MT, P], BF16, tag="wx")
            nc.sync.dma_start(wx_sb[:], wxT.ap().rearrange("p (k m q) -> p k m q", k=KT, m=MT))
            bias_sb = cpool.tile([P, MT], F32, tag="bias")
            nc.sync.dma_start(bias_sb[:], bias[:])
            carry_sb = cpool.tile([P, ROUNDS, KT * BC], mybir.dt.uint8, tag="carry")
            nc.sync.dma_start(carry_sb[:], carry.ap().rearrange("r p c -> p r c"))
            cinit_sb = cpool.tile([P, ROUNDS, KT * BC], BF16, tag="cinit")
            nc.sync.dma_start(cinit_sb[:], cinit.ap().rearrange("r p c -> p r c"))
            gidx_sb = cpool.tile([P, 1], I32, tag="gidx")
            nc.sync.dma_start(gidx_sb[:], gidx[:])
            id_sb = cpool.tile([P, P], BF16, tag="ident")
            nc.sync.dma_start(id_sb[:], ident[:])

            # two persistent block buffers, alternated by round parity
            blkA = blkpool.tile([P, MT, T, BC], BF16, tag="blkA")
            blkB = blkpool.tile([P, MT, T, BC], BF16, tag="blkB")
            nc.vector.memset(blkA[:], 0.0)
            nc.vector.memset(blkB[:], 0.0)

            xw_sb = xwpool.tile([P, MT, T, BC], BF16, tag="xw")

            dx_prev = nc.sync.dma_start(hands[0][4 * P:5 * P, :], x0t[0])
            cc_prev = None
            for r in range(ROUNDS):
                cur = blkA if r % 2 == 0 else blkB
                prev = blkB if r % 2 == 0 else blkA

                # ---- 1. gather input block from this round's board ----
                xblk = xpool.tile([P, KT * T * BC], BF16, tag="xblk")
                g = nc.gpsimd.indirect_dma_start(
                    out=xblk[:],
                    out_offset=None,
                    in_=hands[r][:],
                    in_offset=bass.IndirectOffsetOnAxis(ap=gidx_sb[:, :1], axis=0),
                )
                add_dep_helper(g.ins, dx_prev.ins, sync=True, reason="gather after x-feed")
                if cc_prev is not None:
                    add_dep_helper(g.ins, cc_prev.ins, sync=True, reason="gather after AG")

                # ---- 2. projection: xw[m] = sum_k WxT(k,m).T @ xblk[k] + bias[m]
                # m-pairs interleave two PSUM regions of one bank; bias add
                # fuses the fp32->bf16 cast.
                for mp in range(MT // 2):
                    pp = pppool.tile([P, 2, T, BC], mybir.dt.float32, tag="pp")
                    first_mm = None
                    for k in range(KT):
                        for mi in range(2):
                            m = 2 * mp + mi
                            mm = nc.tensor.matmul(
                                pp[:, mi],
                                wx_sb[:, k, m, :],
                                xblk[:, k * T * BC:(k + 1) * T * BC],
                                start=(k == 0 and mi == 0),
                                stop=(k == KT - 1 and mi == 1),
                                skip_group_check=True,
                            )
                            if first_mm is None:
                                first_mm = mm
                            elif k == 0:
                                add_dep_helper(mm.ins, first_mm.ins, sync=False,
                                               reason="bank clear first")
                    nc.vector.tensor_tensor(
                        out=xw_sb[:, 2 * mp:2 * mp + 2],
                        in0=pp[:],
                        in1=bias_sb[:, 2 * mp:2 * mp + 2, None, None].to_broadcast((P, 2, T, BC)),
                        op=mybir.AluOpType.add,
                    )

                # ---- 3. h_start = carry ? prev_block_tail : cinit ----
                hstart = hspool.tile([P, KT * BC], BF16, tag="hs")
                nc.vector.tensor_copy(hstart[:], cinit_sb[:, r])
                nc.vector.copy_predicated(
                    hstart[:], carry_sb[:, r], prev[:, :, T - 1, :]
                )

                # ---- 4. recurrence over T steps ----
                # One PSUM group per step in a bank-padded tile: the
                # identity matmul seeds xw (start=True clears the bank),
                # 64 wh matmuls accumulate, tanh reads PSUM directly.
                for t in range(T):
                    ps = prpool.tile([P, MT, 64], mybir.dt.float32, tag="ps")
                    idm = nc.tensor.matmul(
                        ps[:, :, 0:BC],
                        id_sb[:],
                        xw_sb[:, :, t, :],
                        start=True,
                        stop=False,
                        skip_group_check=True,
                    )
                    for k in range(KT):
                        if t == 0:
                            rhs = hstart[:, k * BC:(k + 1) * BC]
                        else:
                            rhs = cur[:, k, t - 1, :]
                        for m in range(MT):
                            mm = nc.tensor.matmul(
                                ps[:, m, 0:BC],
                                wh_sb[:, k, m, :],
                                rhs,
                                start=False,
                                stop=(k == KT - 1 and m == MT - 1),
                                skip_group_check=True,
                            )
                            if k == 0:
                                add_dep_helper(mm.ins, idm.ins, sync=False,
                                               reason="bank clear first")
                    nc.scalar.activation(
                        cur[:, 0:4, t, :], ps[:, 0:4, 0:BC], Tanh
                    )
                    nc.scalar.activation(
                        cur[:, 4:8, t, :], ps[:, 4:8, 0:BC], Tanh
                    )

                # ---- 5. write fp32 output block ----
                o32 = opool.tile([P, MT * T * BC], F32, tag="o32")
                nc.vector.tensor_copy(o32[:], cur[:])
                nc.sync.dma_start(out[r], o32[:])

                # ---- 6. handoff: stage block + x-feed, pairwise AGs ----
                if r < ROUNDS - 1:
                    d1 = nc.sync.dma_start(
                        hins[r][:, :],
                        cur[:].rearrange("p m t b -> p (m t b)"),
                    )
                    dx_prev = nc.sync.dma_start(
                        hands[r + 1][4 * P:5 * P, :], x0t[r + 1])
                    cc = nc.gpsimd.collective_compute(
                        "AllGather",
                        mybir.AluOpType.bypass,
                        replica_groups=GROUPS,
                        ins=[hins[r][:]],
                        outs=[hands[r + 1][0:4 * P, :]],
                    )
                    add_dep_helper(cc.ins, d1.ins, sync=True, reason="AG after blk dma")
                    cc_prev = cc
    nc.compile()
    return nc


def _prep_inputs(X, h0s, W, b):
    """Build the 8 per-core input maps."""
    in_maps = []
    for c in range(N_CORES):
        s, j = c // 2, c % 2
        Wl = np.asarray(W[s], dtype=np.float32)
        Wx, Wh = Wl[:, :D], Wl[:, D:]

        def tiles(M):  # M: [e, d] -> lhsT tiles [p, (k, m, q)]
            A = M.reshape(MT, P, KT, P)          # [m, q, k, p]
            return np.ascontiguousarray(
                A.transpose(3, 2, 0, 1).reshape(P, KT * MT * P)).astype(BF)

        whT = tiles(Wh)
        wxT = tiles(Wx)
        bias = np.ascontiguousarray(
            np.asarray(b[s], np.float32).reshape(MT, P).T)

        hin = np.asarray(h0s[s, BC * j:BC * (j + 1)], np.float32)  # [b, d]
        hinit = np.ascontiguousarray(
            hin.reshape(BC, KT, P).transpose(2, 1, 0).reshape(P, KT * BC)).astype(BF)

        carry = np.zeros((ROUNDS, P, KT * BC), np.uint8)
        cinit = np.zeros((ROUNDS, P, KT * BC), BF)
        for r in range(ROUNDS):
            if r > s:
                carry[r] = 1
            else:
                cinit[r] = hinit

        x0t = np.zeros((ROUNDS, P, BLK_COLS), BF)
        if s == 0:
            Xj = np.asarray(X[BC * j:BC * (j + 1)], np.float32)  # [b, L, d]
            # [b, q, t, k, p] -> [q, p, k, t, b]
            Xb = Xj.reshape(BC, NB, T, KT, P).transpose(1, 4, 3, 2, 0)
            Xb = np.ascontiguousarray(Xb.reshape(NB, P, BLK_COLS)).astype(BF)
            x0t[0:NB] = Xb
            gidx = (4 * P + np.arange(P, dtype=np.int32)).reshape(P, 1)
        else:  # stage s reads its predecessor (group position s-1)
            gidx = ((s - 1) * P + np.arange(P, dtype=np.int32)).reshape(P, 1)

        in_maps.append({
            "whT": whT, "wxT": wxT, "bias": bias,
            "carry": carry, "cinit": cinit,
            "gidx": gidx, "ident": np.eye(P, dtype=BF),
            "x0t": x0t,
        })
    return in_maps


def _extract(results):
    """Assemble full output [B, L, D] from stage-3 cores (6, 7)."""
    Y = np.empty((B, L, D), np.float32)
    for j in range(2):
        o = results[6 + j]["out"][NL - 1:NL - 1 + NB]   # [q, p, cols]
        o = o.reshape(NB, P, MT, T, BC).transpose(4, 0, 3, 2, 1)  # [b,q,t,m,p]
        Y[BC * j:BC * (j + 1)] = o.reshape(BC, L, D)
    return Y


def kernel(X, h0s, W, b, _trace=False):
    from concourse.bass_utils import run_bass_kernel_spmd

    if "nc" not in _cache:
        _cache["nc"] = _build()
    nc = _cache["nc"]
    in_maps = _prep_inputs(np.asarray(X), np.asarray(h0s), np.asarray(W),
                           np.asarray(b))
    res = run_bass_kernel_spmd(nc, in_maps, core_ids=list(range(N_CORES)),
                               trace=_trace)
    _cache["last_results"] = res
    return _extract(res.results)


# revision 13
# speedup vs baseline: 1.2980x; 1.1304x over previous
"""4-layer tanh RNN on 8 Trainium2 NeuronCores.

Strategy: 4-stage layer pipeline x 2-way batch split. Core c handles
layer c//2 for batch half c%2. Time is processed in blocks of T=32 steps;
each round every core: gathers its input block (previous stage's output),
projects it (x @ WxT + b), runs 32 recurrence steps (weight-stationary
bf16 matmuls, zT[d_out,b] layout so h never needs a transpose), then
hands its output block to the next stage.

v2 over the original:
- Stage handoff via two pairwise AllGathers (0.5MB->1MB along chain
  edges) instead of one 8-way AllGather (1MB->8MB), writing disjoint
  regions of a combined per-round board tensor; the x-feed for stage-0
  cores rides a third region staged by local DMA. One SPMD-uniform
  indirect gather with a per-core constant row index reads the board.
- xw is accumulated into the step's PSUM group via an identity matmul,
  eliminating the per-step DVE add; tanh reads PSUM directly.
- One PSUM accumulation group per step (all 8 m-tiles, bank-padded
  tile so pool bufs never share a bank), tanh split in m-halves so the
  next step's k<4 matmuls only wait on the first half.
- Projection matmuls interleave m-pairs across two PSUM regions to
  avoid back-to-back same-region accumulation stalls; the bias add is
  fused with the fp32->bf16 cast on DVE.

Compute dtype bf16 (PE fp32 is 4x slower), fp32 PSUM accumulation, fp32
tanh. xw is stored bf16 (feeds the identity matmul).
"""
import sys
import numpy as np

if "/opt/trn_rl_repo" not in sys.path:
    sys.path.insert(0, "/opt/trn_rl_repo")

import ml_dtypes

BF = ml_dtypes.bfloat16

# Problem config (hardcoded per contract)
import os as _os
B, L, D, NL = 16, 512, 1024, 4
P = 128
KT = D // P          # 8 k-tiles (contraction)
MT = D // P          # 8 m-tiles (output)
BC = B // 2          # 8 = per-core batch half
T = 32               # timesteps per block
NB = L // T          # 16 blocks
ROUNDS = NB + NL - 1  # 19
N_CORES = 8
BLK_COLS = MT * T * BC  # 2048 block columns: col = m*T*BC + t*BC + b

# one chain per batch half: stage s core is group position s, so each
# receiver reads its predecessor's block at rows (s-1)*P of the AG out.
# (The runtime rejects NEFFs mixing two different replica-group sets,
# so the chain edges ride one 4-core AllGather: 2MB out/core vs 8MB
# for the 8-way gather.)
GROUPS = [[0, 2, 4, 6], [1, 3, 5, 7]]

_cache = {}


def _build():
    import concourse.bass as bass
    import concourse.mybir as mybir
    import concourse.tile as tile
    from concourse import bacc
    from concourse.tile import add_dep_helper

    F32 = mybir.dt.float32
    BF16 = mybir.dt.bfloat16
    I32 = mybir.dt.int32
    Tanh = mybir.ActivationFunctionType.Tanh

    nc = bacc.Bacc("TRN2", target_bir_lowering=False, debug=False,
                   num_devices=N_CORES)

    # ---- I/O ----
    whT = nc.dram_tensor("whT", [P, KT * MT * P], BF16, kind="ExternalInput")
    wxT = nc.dram_tensor("wxT", [P, KT * MT * P], BF16, kind="ExternalInput")
    bias = nc.dram_tensor("bias", [P, MT], F32, kind="ExternalInput")
    carry = nc.dram_tensor("carry", [ROUNDS, P, KT * BC], mybir.dt.uint8, kind="ExternalInput")
    cinit = nc.dram_tensor("cinit", [ROUNDS, P, KT * BC], BF16, kind="ExternalInput")
    gidx = nc.dram_tensor("gidx", [P, 1], I32, kind="ExternalInput")
    ident = nc.dram_tensor("ident", [P, P], BF16, kind="ExternalInput")
    x0t = nc.dram_tensor("x0t", [ROUNDS, P, BLK_COLS], BF16, kind="ExternalInput")
    out = nc.dram_tensor("out", [ROUNDS, P, BLK_COLS], F32, kind="ExternalOutput")

    # handoff board per round: rows [0:4P) AllGather out (one row-block
    # per chain stage), [4P:5P) x-feed (staged by local DMA from x0t).
    # Local (non-shared) outputs: 2-core groups don't support the shared
    # scratchpad, and AllGather lands each group's blocks in every
    # member's local copy anyway, which is all the gather reads.
    hands = [nc.dram_tensor(f"hand_{r}", [5 * P, BLK_COLS], BF16)
             for r in range(ROUNDS)]
    hins = [nc.dram_tensor(f"hin_{r}", [P, BLK_COLS], BF16)
            for r in range(ROUNDS - 1)]

    with tile.TileContext(nc) as tc:
        with (
            tc.tile_pool(name="const", bufs=1) as cpool,
            tc.tile_pool(name="xblk", bufs=2) as xpool,
            tc.tile_pool(name="xw", bufs=1) as xwpool,
            tc.tile_pool(name="blk", bufs=1) as blkpool,
            tc.tile_pool(name="hs", bufs=2) as hspool,
            tc.tile_pool(name="o32", bufs=2) as opool,
            tc.tile_pool(name="psr", bufs=2, space="PSUM") as prpool,
            tc.tile_pool(name="psp", bufs=2, space="PSUM") as pppool,
        ):
            wh_sb = cpool.tile([P, KT, MT, P], BF16, tag="wh")
            nc.sync.dma_start(wh_sb[:], whT.ap().rearrange("p (k m q) -> p k m q", k=KT, m=MT))
            wx_sb = cpool.tile([P, KT, MT, P], BF16, tag="wx")
            nc.sync.dma_start(wx_sb[:], wxT.ap().rearrange("p (k m q) -> p k m q", k=KT, m=MT))
            bias_sb = cpool.tile([P, MT], F32, tag="bias")
            nc.sync.dma_start(bias_sb[:], bias[:])
            carry_sb = cpool.tile([P, ROUNDS, KT * BC], mybir.dt.uint8, tag="carry")
            nc.sync.dma_start(carry_sb[:], carry.ap().rearrange("r p c -> p r c"))
            cinit_sb = cpool.tile([P, ROUNDS, KT * BC], BF16, tag="cinit")
            nc.sync.dma_start(cinit_sb[:], cinit.ap().rearrange("r p c -> p r c"))
            gidx_sb = cpool.tile([P, 1], I32, tag="gidx")
            nc.sync.dma_start(gidx_sb[:], gidx[:])
            id_sb = cpool.tile([P, P], BF16, tag="ident")
            nc.sync.dma_start(id_sb[:], ident[:])

            # two persistent block buffers, alternated by round parity
            blkA = blkpool.tile([P, MT, T, BC], BF16, tag="blkA")
            blkB = blkpool.tile([P, MT, T, BC], BF16, tag="blkB")
            nc.vector.memset(blkA[:], 0.0)
            nc.vector.memset(blkB[:], 0.0)

            xw_sb = xwpool.tile([P, MT, T, BC], BF16, tag="xw")

            dx_prev = nc.sync.dma_start(hands[0][4 * P:5 * P, :], x0t[0])
            cc_prev = None
            for r in range(ROUNDS):
                cur = blkA if r % 2 == 0 else blkB
                prev = blkB if r % 2 == 0 else blkA

                # ---- 1. gather input block from this round's board ----
                xblk = xpool.tile([P, KT * T * BC], BF16, tag="xblk")
                g = nc.gpsimd.indirect_dma_start(
                    out=xblk[:],
                    out_offset=None,
                    in_=hands[r][:],
                    in_offset=bass.IndirectOffsetOnAxis(ap=gidx_sb[:, :1], axis=0),
                )
                add_dep_helper(g.ins, dx_prev.ins, sync=True, reason="gather after x-feed")
                if cc_prev is not None:
                    add_dep_helper(g.ins, cc_prev.ins, sync=True, reason="gather after AG")

                # ---- 2. projection: xw[m] = sum_k WxT(k,m).T @ xblk[k] + bias[m]
                # m-pairs interleave two PSUM regions of one bank; bias add
                # fuses the fp32->bf16 cast.
                for mp in range(MT // 2):
                    pp = pppool.tile([P, 2, T, BC], mybir.dt.float32, tag="pp")
                    first_mm = None
                    for k in range(KT):
                        for mi in range(2):
                            m = 2 * mp + mi
                            mm = nc.tensor.matmul(
                                pp[:, mi],
                                wx_sb[:, k, m, :],
                                xblk[:, k * T * BC:(k + 1) * T * BC],
                                start=(k == 0 and mi == 0),
                                stop=(k == KT - 1 and mi == 1),
                                skip_group_check=True,
                            )
                            if first_mm is None:
                                first_mm = mm
                            elif k == 0:
                                add_dep_helper(mm.ins, first_mm.ins, sync=False,
                                               reason="bank clear first")
                    nc.vector.tensor_tensor(
                        out=xw_sb[:, 2 * mp:2 * mp + 2],
                        in0=pp[:],
                        in1=bias_sb[:, 2 * mp:2 * mp + 2, None, None].to_broadcast((P, 2, T, BC)),
                        op=mybir.AluOpType.add,
                    )

                # ---- 3. h_start = carry ? prev_block_tail : cinit ----
                hstart = hspool.tile([P, KT * BC], BF16, tag="hs")
                nc.vector.tensor_copy(hstart[:], cinit_sb[:, r])
                nc.vector.copy_predicated(
                    hstart[:], carry_sb[:, r], prev[:, :, T - 1, :]
                )

                # ---- 4. recurrence over T steps ----
                # Two PSUM groups per step (m-halves), each in its own
                # bank-padded tile: an identity matmul seeds xw
                # (start=True clears the bank), 32 wh matmuls
                # accumulate, tanh reads PSUM directly as soon as its
                # half's group stops -- so the first half's tanh
                # overlaps the second half's matmuls and the next
                # step's k<4 matmuls only wait on the first tanh.
                for t in range(T):
                    pslo = prpool.tile([P, 4, 128], mybir.dt.float32, tag="pslo")
                    pshi = prpool.tile([P, 4, 128], mybir.dt.float32, tag="pshi")
                    idlo = nc.tensor.matmul(
                        pslo[:, :, 0:BC], id_sb[:], xw_sb[:, 0:4, t, :],
                        start=True, stop=False, skip_group_check=True,
                    )
                    idhi = nc.tensor.matmul(
                        pshi[:, :, 0:BC], id_sb[:], xw_sb[:, 4:8, t, :],
                        start=True, stop=False, skip_group_check=True,
                    )
                    for half in range(2):
                        ps = pslo if half == 0 else pshi
                        idm = idlo if half == 0 else idhi
                        for k in range(KT):
                            if t == 0:
                                rhs = hstart[:, k * BC:(k + 1) * BC]
                            else:
                                rhs = cur[:, k, t - 1, :]
                            for mi in range(4):
                                m = 4 * half + mi
                                mm = nc.tensor.matmul(
                                    ps[:, mi, 0:BC],
                                    wh_sb[:, k, m, :],
                                    rhs,
                                    start=False,
                                    stop=(k == KT - 1 and mi == 3),
                                    skip_group_check=True,
                                )
                                if k == 0:
                                    add_dep_helper(mm.ins, idm.ins, sync=False,
                                                   reason="bank clear first")
                        nc.scalar.activation(
                            cur[:, 4 * half:4 * half + 4, t, :],
                            ps[:, :, 0:BC], Tanh
                        )

                # ---- 5. write fp32 output block ----
                o32 = opool.tile([P, MT * T * BC], F32, tag="o32")
                nc.vector.tensor_copy(o32[:], cur[:])
                nc.sync.dma_start(out[r], o32[:])

                # ---- 6. handoff: stage block + x-feed, pairwise AGs ----
                if r < ROUNDS - 1:
                    d1 = nc.sync.dma_start(
                        hins[r][:, :],
                        cur[:].rearrange("p m t b -> p (m t b)"),
                    )
                    dx_prev = nc.sync.dma_start(
                        hands[r + 1][4 * P:5 * P, :], x0t[r + 1])
                    cc = nc.gpsimd.collective_compute(
                        "AllGather",
                        mybir.AluOpType.bypass,
                        replica_groups=GROUPS,
                        ins=[hins[r][:]],
                        outs=[hands[r + 1][0:4 * P, :]],
                    )
                    add_dep_helper(cc.ins, d1.ins, sync=True, reason="AG after blk dma")
                    cc_prev = cc
    nc.compile()
    return nc


def _prep_inputs(X, h0s, W, b):
    """Build the 8 per-core input maps."""
    in_maps = []
    for c in range(N_CORES):
        s, j = c // 2, c % 2
        Wl = np.asarray(W[s], dtype=np.float32)
        Wx, Wh = Wl[:, :D], Wl[:, D:]

        def tiles(M):  # M: [e, d] -> lhsT tiles [p, (k, m, q)]
            A = M.reshape(MT, P, KT, P)          # [m, q, k, p]
            return np.ascontiguousarray(
                A.transpose(3, 2, 0, 1).reshape(P, KT * MT * P)).astype(BF)

        whT = tiles(Wh)
        wxT = tiles(Wx)
        bias = np.ascontiguousarray(
            np.asarray(b[s], np.float32).reshape(MT, P).T)

        hin = np.asarray(h0s[s, BC * j:BC * (j + 1)], np.float32)  # [b, d]
        hinit = np.ascontiguousarray(
            hin.reshape(BC, KT, P).transpose(2, 1, 0).reshape(P, KT * BC)).astype(BF)

        carry = np.zeros((ROUNDS, P, KT * BC), np.uint8)
        cinit = np.zeros((ROUNDS, P, KT * BC), BF)
        for r in range(ROUNDS):
            if r > s:
                carry[r] = 1
            else:
                cinit[r] = hinit

        x0t = np.zeros((ROUNDS, P, BLK_COLS), BF)
        if s == 0:
            Xj = np.asarray(X[BC * j:BC * (j + 1)], np.float32)  # [b, L, d]
            # [b, q, t, k, p] -> [q, p, k, t, b]
            Xb = Xj.reshape(BC, NB, T, KT, P).transpose(1, 4, 3, 2, 0)
            Xb = np.ascontiguousarray(Xb.reshape(NB, P, BLK_COLS)).astype(BF)
            x0t[0:NB] = Xb
            gidx = (4 * P + np.arange(P, dtype=np.int32)).reshape(P, 1)
        else:  # stage s reads its predecessor (group position s-1)
            gidx = ((s - 1) * P + np.arange(P, dtype=np.int32)).reshape(P, 1)

        in_maps.append({
            "whT": whT, "wxT": wxT, "bias": bias,
            "carry": carry, "cinit": cinit,
            "gidx": gidx, "ident": np.eye(P, dtype=BF),
            "x0t": x0t,
        })
    return in_maps


def _extract(results):
    """Assemble full output [B, L, D] from stage-3 cores (6, 7)."""
    Y = np.empty((B, L, D), np.float32)
    for j in range(2):
        o = results[6 + j]["out"][NL - 1:NL - 1 + NB]   # [q, p, cols]
        o = o.reshape(NB, P, MT, T, BC).transpose(4, 0, 3, 2, 1)  # [b,q,t,m,p]
        Y[BC * j:BC * (j + 1)] = o.reshape(BC, L, D)
    return Y


def kernel(X, h0s, W, b, _trace=False):
    from concourse.bass_utils import run_bass_kernel_spmd

    if "nc" not in _cache:
        _cache["nc"] = _build()
    nc = _cache["nc"]
    in_maps = _prep_inputs(np.asarray(X), np.asarray(h0s), np.asarray(W),
                           np.asarray(b))
    res = run_bass_kernel_spmd(nc, in_maps, core_ids=list(range(N_CORES)),
                               trace=_trace)
    _cache["last_results"] = res
    return _extract(res.results)


# revision 16
# speedup vs baseline: 1.4491x; 1.1165x over previous
"""4-layer tanh RNN on 8 Trainium2 NeuronCores.

Strategy: 4-stage layer pipeline x 2-way batch split. Core c handles
layer c//2 for batch half c%2. Time is processed in blocks of T=32
steps; each round every core gathers its input block (previous stage's
output), projects it (x @ WxT + b), runs 32 recurrence steps
(weight-stationary bf16 matmuls, zT[d_out,b] layout so h never needs a
transpose), then hands its output block to the next stage.

Pipelining (v4): the handoff is split into half-blocks. The first half
(t<16) ships mid-round, so its AllGather and the successor's gather +
projection all overlap the producer's second half; the second half
ships at round end and is projected by the consumer mid-next-round.
Projection matmuls are interleaved a few per recurrence step, riding
the PE idle gaps the per-step tanh dependency creates, so projection
costs almost no wall-clock. Stage handoff uses one 4-core-group
AllGather per half along each batch-half chain (group position = stage,
2x0.5MB out/core/round vs 8MB for a flat 8-way gather); stage-0 cores
read their x-feed from a board region staged by local DMA. One
SPMD-uniform indirect gather with a per-core constant row index reads
the board.

Recurrence step: two PSUM groups (m-halves) in bank-padded tiles; an
identity matmul seeds xw into each group (start=True clears the bank),
32 wh matmuls accumulate, tanh reads PSUM directly as soon as its
half's group stops.

Compute dtype bf16 (PE fp32 is 4x slower), fp32 PSUM accumulation, fp32
tanh. xw is stored bf16 (it feeds the identity matmul).
"""
import sys
import numpy as np

if "/opt/trn_rl_repo" not in sys.path:
    sys.path.insert(0, "/opt/trn_rl_repo")

import ml_dtypes

BF = ml_dtypes.bfloat16

# Problem config (hardcoded per contract)
import os as _os
B, L, D, NL = 16, 512, 1024, 4
P = 128
KT = D // P          # 8 k-tiles (contraction)
MT = D // P          # 8 m-tiles (output)
BC = B // 2          # 8 = per-core batch half
T = 32               # timesteps per block
TH = T // 2          # 16 = half-block timesteps
NB = L // T          # 16 blocks
ROUNDS = NB + NL - 1  # 19
N_CORES = 8
BLK_COLS = MT * T * BC   # 2048 block columns
HALF_COLS = MT * TH * BC  # 1024 half-block columns: col = m*TH*BC + tl*BC + b

# one chain per batch half: stage s core is group position s, so each
# receiver reads its predecessor's block at rows (s-1)*P of the AG out.
GROUPS = [[0, 2, 4, 6], [1, 3, 5, 7]]

_cache = {}


def _build():
    import concourse.bass as bass
    import concourse.mybir as mybir
    import concourse.tile as tile
    from concourse import bacc
    from concourse.tile import add_dep_helper

    F32 = mybir.dt.float32
    BF16 = mybir.dt.bfloat16
    I32 = mybir.dt.int32
    Tanh = mybir.ActivationFunctionType.Tanh

    nc = bacc.Bacc("TRN2", target_bir_lowering=False, debug=False,
                   num_devices=N_CORES)

    # ---- I/O ----
    whT = nc.dram_tensor("whT", [P, KT * MT * P], BF16, kind="ExternalInput")
    wxT = nc.dram_tensor("wxT", [P, KT * MT * P], BF16, kind="ExternalInput")
    bias = nc.dram_tensor("bias", [P, MT], F32, kind="ExternalInput")
    carry = nc.dram_tensor("carry", [ROUNDS, P, KT * BC], mybir.dt.uint8, kind="ExternalInput")
    cinit = nc.dram_tensor("cinit", [ROUNDS, P, KT * BC], BF16, kind="ExternalInput")
    gidx = nc.dram_tensor("gidx", [P, 1], I32, kind="ExternalInput")
    ident = nc.dram_tensor("ident", [P, P], BF16, kind="ExternalInput")
    # x-feed, halved column layout: col = h*HALF_COLS + k*TH*BC + tl*BC + b
    x0t = nc.dram_tensor("x0t", [ROUNDS, P, BLK_COLS], BF16, kind="ExternalInput")
    out = nc.dram_tensor("out", [ROUNDS, P, BLK_COLS], F32, kind="ExternalOutput")

    # handoff boards, one per half-block per round: rows [0:4P) AllGather
    # out (one row-block per chain stage), [4P:5P) x-feed staged by DMA.
    hands = [[nc.dram_tensor(f"hand{h}_{r}", [5 * P, HALF_COLS], BF16)
              for r in range(ROUNDS)] for h in range(2)]
    hins = [[nc.dram_tensor(f"hin{h}_{r}", [P, HALF_COLS], BF16)
             for r in range(ROUNDS - 1)] for h in range(2)]

    with tile.TileContext(nc) as tc:
        with (
            tc.tile_pool(name="const", bufs=1) as cpool,
            tc.tile_pool(name="xw", bufs=1) as xwpool,
            tc.tile_pool(name="blk", bufs=1) as blkpool,
            tc.tile_pool(name="hs", bufs=2) as hspool,
            tc.tile_pool(name="o32", bufs=2) as opool,
            tc.tile_pool(name="psr", bufs=2, space="PSUM") as prpool,
            tc.tile_pool(name="psp", bufs=2, space="PSUM") as pppool,
        ):
            wh_sb = cpool.tile([P, KT, MT, P], BF16, tag="wh")
            nc.sync.dma_start(wh_sb[:], whT.ap().rearrange("p (k m q) -> p k m q", k=KT, m=MT))
            wx_sb = cpool.tile([P, KT, MT, P], BF16, tag="wx")
            nc.sync.dma_start(wx_sb[:], wxT.ap().rearrange("p (k m q) -> p k m q", k=KT, m=MT))
            bias_sb = cpool.tile([P, MT], F32, tag="bias")
            nc.sync.dma_start(bias_sb[:], bias[:])
            carry_sb = cpool.tile([P, ROUNDS, KT * BC], mybir.dt.uint8, tag="carry")
            nc.sync.dma_start(carry_sb[:], carry.ap().rearrange("r p c -> p r c"))
            cinit_sb = cpool.tile([P, ROUNDS, KT * BC], BF16, tag="cinit")
            nc.sync.dma_start(cinit_sb[:], cinit.ap().rearrange("r p c -> p r c"))
            gidx_sb = cpool.tile([P, 1], I32, tag="gidx")
            nc.sync.dma_start(gidx_sb[:], gidx[:])
            id_sb = cpool.tile([P, P], BF16, tag="ident")
            nc.sync.dma_start(id_sb[:], ident[:])

            # persistent double buffers, alternated by round parity
            blkA = blkpool.tile([P, MT, T, BC], BF16, tag="blkA")
            blkB = blkpool.tile([P, MT, T, BC], BF16, tag="blkB")
            nc.vector.memset(blkA[:], 0.0)
            nc.vector.memset(blkB[:], 0.0)
            # gathered input block, cols (h, k, tl, b)
            xbA = blkpool.tile([P, 2, HALF_COLS], BF16, tag="xbA")
            xbB = blkpool.tile([P, 2, HALF_COLS], BF16, tag="xbB")

            xw_sb = xwpool.tile([P, MT, T, BC], BF16, tag="xw")

            def gather(xb, h, r, deps):
                g = nc.gpsimd.indirect_dma_start(
                    out=xb[:, h, :],
                    out_offset=None,
                    in_=hands[h][r][:],
                    in_offset=bass.IndirectOffsetOnAxis(ap=gidx_sb[:, :1], axis=0),
                )
                for dep, reason in deps:
                    add_dep_helper(g.ins, dep.ins, sync=True, reason=reason)
                return g

            def proj_pair(xb, h, mp):
                """One m-pair PSUM group of the projection for half h."""
                pp = pppool.tile([P, 2, TH, 16], mybir.dt.float32, tag="pp")
                first_mm = None
                for k in range(KT):
                    for mi in range(2):
                        m = 2 * mp + mi
                        mm = nc.tensor.matmul(
                            pp[:, mi, :, 0:BC],
                            wx_sb[:, k, m, :],
                            xb[:, h, k * TH * BC:(k + 1) * TH * BC],
                            start=(k == 0 and mi == 0),
                            stop=(k == KT - 1 and mi == 1),
                            skip_group_check=True,
                        )
                        if first_mm is None:
                            first_mm = mm
                        elif k == 0:
                            add_dep_helper(mm.ins, first_mm.ins, sync=False,
                                           reason="bank clear first")
                nc.vector.tensor_tensor(
                    out=xw_sb[:, 2 * mp:2 * mp + 2, h * TH:(h + 1) * TH, :],
                    in0=pp[:, :, :, 0:BC],
                    in1=bias_sb[:, 2 * mp:2 * mp + 2, None, None].to_broadcast((P, 2, TH, BC)),
                    op=mybir.AluOpType.add,
                )

            def rec_step(cur, hstart, t):
                pslo = prpool.tile([P, 4, 128], mybir.dt.float32, tag="pslo")
                pshi = prpool.tile([P, 4, 128], mybir.dt.float32, tag="pshi")
                idlo = nc.tensor.matmul(
                    pslo[:, :, 0:BC], id_sb[:], xw_sb[:, 0:4, t, :],
                    start=True, stop=False, skip_group_check=True,
                )
                idhi = nc.tensor.matmul(
                    pshi[:, :, 0:BC], id_sb[:], xw_sb[:, 4:8, t, :],
                    start=True, stop=False, skip_group_check=True,
                )
                for half in range(2):
                    ps = pslo if half == 0 else pshi
                    idm = idlo if half == 0 else idhi
                    for k in range(KT):
                        if t == 0:
                            rhs = hstart[:, k * BC:(k + 1) * BC]
                        else:
                            rhs = cur[:, k, t - 1, :]
                        for mi in range(4):
                            m = 4 * half + mi
                            mm = nc.tensor.matmul(
                                ps[:, mi, 0:BC],
                                wh_sb[:, k, m, :],
                                rhs,
                                start=False,
                                stop=(k == KT - 1 and mi == 3),
                                skip_group_check=True,
                            )
                            if k == 0:
                                add_dep_helper(mm.ins, idm.ins, sync=False,
                                               reason="bank clear first")
                    nc.scalar.activation(
                        cur[:, 4 * half:4 * half + 4, t, :],
                        ps[:, :, 0:BC], Tanh
                    )

            # stage round-0 x-feeds
            dx0 = nc.sync.dma_start(hands[0][0][4 * P:5 * P, :], x0t[0][:, 0:HALF_COLS])
            dx1 = nc.sync.dma_start(hands[1][0][4 * P:5 * P, :], x0t[0][:, HALF_COLS:])
            dx_prev = [dx0, dx1]

            cc_h0 = cc_h1 = None
            for r in range(ROUNDS):
                cur = blkA if r % 2 == 0 else blkB
                prev = blkB if r % 2 == 0 else blkA
                xb = xbA if r % 2 == 0 else xbB
                xb_next = xbB if r % 2 == 0 else xbA

                if r == 0:
                    # no prior round overlapped this work: gather+project
                    # the first half up front.
                    g0 = gather(xb, 0, 0, [(dx_prev[0], "gather after x-feed")])
                    for mp in range(4):
                        proj_pair(xb, 0, mp)

                # second-half gather for this round (its AG launched at the
                # end of round r-1; projection is interleaved into steps
                # 8..15 below).
                deps = [(dx_prev[1], "gather after x-feed")]
                if cc_h1 is not None:
                    deps.append((cc_h1, "gather after AG"))
                gather(xb, 1, r, deps)

                # h_start = carry ? prev_block_tail : cinit
                hstart = hspool.tile([P, KT * BC], BF16, tag="hs")
                nc.vector.tensor_copy(hstart[:], cinit_sb[:, r])
                nc.vector.copy_predicated(
                    hstart[:], carry_sb[:, r], prev[:, :, T - 1, :]
                )

                for t in range(T):
                    rec_step(cur, hstart, t)

                    # interleave projections into the tanh-latency gaps:
                    # steps 8..11 project this round's second half (one
                    # m-pair per step), steps 24..27 the next round's
                    # first half (gathered below after its mid-round AG).
                    if 8 <= t < 12:
                        proj_pair(xb, 1, t - 8)
                    if 24 <= t < 28 and r < ROUNDS - 1:
                        proj_pair(xb_next, 0, t - 24)

                    if t == TH - 1 and r < ROUNDS - 1:
                        # ship the first half-block mid-round
                        d1 = nc.sync.dma_start(
                            hins[0][r].ap().rearrange("p (m t b) -> p m t b", m=MT, t=TH),
                            cur[:, :, 0:TH, :],
                        )
                        dxn0 = nc.sync.dma_start(
                            hands[0][r + 1][4 * P:5 * P, :],
                            x0t[r + 1][:, 0:HALF_COLS])
                        cc_h0 = nc.gpsimd.collective_compute(
                            "AllGather",
                            mybir.AluOpType.bypass,
                            replica_groups=GROUPS,
                            ins=[hins[0][r][:]],
                            outs=[hands[0][r + 1][0:4 * P, :]],
                        )
                        add_dep_helper(cc_h0.ins, d1.ins, sync=True,
                                       reason="AG after blk dma")
                        gather(xb_next, 0, r + 1,
                               [(cc_h0, "gather after AG"),
                                (dxn0, "gather after x-feed")])

                # write fp32 output block
                o32 = opool.tile([P, MT * T * BC], F32, tag="o32")
                nc.vector.tensor_copy(o32[:], cur[:])
                nc.sync.dma_start(out[r], o32[:])

                # ship the second half-block at round end
                if r < ROUNDS - 1:
                    d2 = nc.sync.dma_start(
                        hins[1][r].ap().rearrange("p (m t b) -> p m t b", m=MT, t=TH),
                        cur[:, :, TH:T, :],
                    )
                    dxn1 = nc.sync.dma_start(
                        hands[1][r + 1][4 * P:5 * P, :],
                        x0t[r + 1][:, HALF_COLS:])
                    cc_h1 = nc.gpsimd.collective_compute(
                        "AllGather",
                        mybir.AluOpType.bypass,
                        replica_groups=GROUPS,
                        ins=[hins[1][r][:]],
                        outs=[hands[1][r + 1][0:4 * P, :]],
                    )
                    add_dep_helper(cc_h1.ins, d2.ins, sync=True,
                                   reason="AG after blk dma")
                    dx_prev = [dxn0, dxn1]
    nc.compile()
    return nc


def _prep_inputs(X, h0s, W, b):
    """Build the 8 per-core input maps."""
    in_maps = []
    for c in range(N_CORES):
        s, j = c // 2, c % 2
        Wl = np.asarray(W[s], dtype=np.float32)
        Wx, Wh = Wl[:, :D], Wl[:, D:]

        def tiles(M):  # M: [e, d] -> lhsT tiles [p, (k, m, q)]
            A = M.reshape(MT, P, KT, P)          # [m, q, k, p]
            return np.ascontiguousarray(
                A.transpose(3, 2, 0, 1).reshape(P, KT * MT * P)).astype(BF)

        whT = tiles(Wh)
        wxT = tiles(Wx)
        bias = np.ascontiguousarray(
            np.asarray(b[s], np.float32).reshape(MT, P).T)

        hin = np.asarray(h0s[s, BC * j:BC * (j + 1)], np.float32)  # [b, d]
        hinit = np.ascontiguousarray(
            hin.reshape(BC, KT, P).transpose(2, 1, 0).reshape(P, KT * BC)).astype(BF)

        carry = np.zeros((ROUNDS, P, KT * BC), np.uint8)
        cinit = np.zeros((ROUNDS, P, KT * BC), BF)
        for r in range(ROUNDS):
            if r > s:
                carry[r] = 1
            else:
                cinit[r] = hinit

        x0t = np.zeros((ROUNDS, P, BLK_COLS), BF)
        if s == 0:
            Xj = np.asarray(X[BC * j:BC * (j + 1)], np.float32)  # [b, L, d]
            # [b, blk, h, tl, k, p] -> [blk, p, h, k, tl, b]
            Xb = Xj.reshape(BC, NB, 2, TH, KT, P).transpose(1, 5, 2, 4, 3, 0)
            Xb = np.ascontiguousarray(Xb.reshape(NB, P, BLK_COLS)).astype(BF)
            x0t[0:NB] = Xb
            gidx = (4 * P + np.arange(P, dtype=np.int32)).reshape(P, 1)
        else:  # stage s reads its predecessor (group position s-1)
            gidx = ((s - 1) * P + np.arange(P, dtype=np.int32)).reshape(P, 1)

        in_maps.append({
            "whT": whT, "wxT": wxT, "bias": bias,
            "carry": carry, "cinit": cinit,
            "gidx": gidx, "ident": np.eye(P, dtype=BF),
            "x0t": x0t,
        })
    return in_maps


def _extract(results):
    """Assemble full output [B, L, D] from stage-3 cores (6, 7)."""
    Y = np.empty((B, L, D), np.float32)
    for j in range(2):
        o = results[6 + j]["out"][NL - 1:NL - 1 + NB]   # [q, p, cols]
        o = o.reshape(NB, P, MT, T, BC).transpose(4, 0, 3, 2, 1)  # [b,q,t,m,p]
        Y[BC * j:BC * (j + 1)] = o.reshape(BC, L, D)
    return Y


def kernel(X, h0s, W, b, _trace=False):
    from concourse.bass_utils import run_bass_kernel_spmd

    if "nc" not in _cache:
        _cache["nc"] = _build()
    nc = _cache["nc"]
    in_maps = _prep_inputs(np.asarray(X), np.asarray(h0s), np.asarray(W),
                           np.asarray(b))
    res = run_bass_kernel_spmd(nc, in_maps, core_ids=list(range(N_CORES)),
                               trace=_trace)
    _cache["last_results"] = res
    return _extract(res.results)


# revision 21
# speedup vs baseline: 1.5082x; 1.0407x over previous
"""4-layer tanh RNN on 8 Trainium2 NeuronCores.

Strategy: 4-stage layer pipeline x 2-way batch split. Core c handles
layer c//2 for batch half c%2. Time is processed in blocks of T=32
steps; each round every core gathers its input block (previous stage's
output), projects it (x @ WxT + b), runs 32 recurrence steps
(weight-stationary bf16 matmuls, zT[d_out,b] layout so h never needs a
transpose), then hands its output block to the next stage.

Pipelining (v4): the handoff is split into half-blocks. The first half
(t<16) ships mid-round, so its AllGather and the successor's gather +
projection all overlap the producer's second half; the second half
ships at round end and is projected by the consumer mid-next-round.
Projection matmuls are interleaved a few per recurrence step, riding
the PE idle gaps the per-step tanh dependency creates, so projection
costs almost no wall-clock. Stage handoff uses one 4-core-group
AllGather per half along each batch-half chain (group position = stage,
2x0.5MB out/core/round vs 8MB for a flat 8-way gather); stage-0 cores
read their x-feed from a board region staged by local DMA. One
SPMD-uniform indirect gather with a per-core constant row index reads
the board.

Recurrence step: two PSUM groups (m-halves) in bank-padded tiles; an
identity matmul seeds xw into each group (start=True clears the bank),
32 wh matmuls accumulate, tanh reads PSUM directly as soon as its
half's group stops.

Compute dtype bf16 (PE fp32 is 4x slower), fp32 PSUM accumulation, fp32
tanh. xw is stored bf16 (it feeds the identity matmul).
"""
import sys
import numpy as np

if "/opt/trn_rl_repo" not in sys.path:
    sys.path.insert(0, "/opt/trn_rl_repo")

import ml_dtypes

BF = ml_dtypes.bfloat16

# Problem config (hardcoded per contract)
import os as _os
B, L, D, NL = 16, 512, 1024, 4
P = 128
KT = D // P          # 8 k-tiles (contraction)
MT = D // P          # 8 m-tiles (output)
BC = B // 2          # 8 = per-core batch half
T = 32               # timesteps per block
TH = T // 2          # 16 = half-block timesteps
NB = L // T          # 16 blocks
ROUNDS = NB + NL - 1  # 19
N_CORES = 8
BLK_COLS = MT * T * BC   # 2048 block columns
HALF_COLS = MT * TH * BC  # 1024 half-block columns: col = m*TH*BC + tl*BC + b

# one chain per batch half: stage s core is group position s, so each
# receiver reads its predecessor's block at rows (s-1)*P of the AG out.
GROUPS = [[0, 2, 4, 6], [1, 3, 5, 7]]

_cache = {}


def _build():
    import concourse.bass as bass
    import concourse.mybir as mybir
    import concourse.tile as tile
    from concourse import bacc
    from concourse.tile import add_dep_helper

    F32 = mybir.dt.float32
    BF16 = mybir.dt.bfloat16
    I32 = mybir.dt.int32
    Tanh = mybir.ActivationFunctionType.Tanh

    nc = bacc.Bacc("TRN2", target_bir_lowering=False, debug=False,
                   num_devices=N_CORES)

    # ---- I/O ----
    whT = nc.dram_tensor("whT", [P, KT * MT * P], BF16, kind="ExternalInput")
    wxT = nc.dram_tensor("wxT", [P, KT * MT * P], BF16, kind="ExternalInput")
    bias = nc.dram_tensor("bias", [P, MT], F32, kind="ExternalInput")
    carry = nc.dram_tensor("carry", [ROUNDS, P, KT * BC], mybir.dt.uint8, kind="ExternalInput")
    cinit = nc.dram_tensor("cinit", [ROUNDS, P, KT * BC], BF16, kind="ExternalInput")
    gidx = nc.dram_tensor("gidx", [P, 1], I32, kind="ExternalInput")
    ident = nc.dram_tensor("ident", [P, P], BF16, kind="ExternalInput")
    # x-feed, halved column layout: col = h*HALF_COLS + k*TH*BC + tl*BC + b
    x0t = nc.dram_tensor("x0t", [ROUNDS, P, BLK_COLS], BF16, kind="ExternalInput")
    out = nc.dram_tensor("out", [ROUNDS, P, BLK_COLS], F32, kind="ExternalOutput")

    # handoff boards, one per half-block per round: rows [0:4P) AllGather
    # out (one row-block per chain stage), [4P:5P) x-feed staged by DMA.
    hands = [[nc.dram_tensor(f"hand{h}_{r}", [5 * P, HALF_COLS], BF16)
              for r in range(ROUNDS)] for h in range(2)]
    hins = [[nc.dram_tensor(f"hin{h}_{r}", [P, HALF_COLS], BF16)
             for r in range(ROUNDS - 1)] for h in range(2)]

    with tile.TileContext(nc) as tc:
        with (
            tc.tile_pool(name="const", bufs=1) as cpool,
            tc.tile_pool(name="xw", bufs=1) as xwpool,
            tc.tile_pool(name="blk", bufs=1) as blkpool,
            tc.tile_pool(name="hs", bufs=2) as hspool,
            tc.tile_pool(name="o32", bufs=2) as opool,
            tc.tile_pool(name="psr", bufs=2, space="PSUM") as prpool,
            tc.tile_pool(name="psp", bufs=2, space="PSUM") as pppool,
        ):
            wh_sb = cpool.tile([P, KT, MT, P], BF16, tag="wh")
            nc.sync.dma_start(wh_sb[:], whT.ap().rearrange("p (k m q) -> p k m q", k=KT, m=MT))
            wx_sb = cpool.tile([P, KT, MT, P], BF16, tag="wx")
            nc.sync.dma_start(wx_sb[:], wxT.ap().rearrange("p (k m q) -> p k m q", k=KT, m=MT))
            bias_sb = cpool.tile([P, MT], F32, tag="bias")
            nc.sync.dma_start(bias_sb[:], bias[:])
            carry_sb = cpool.tile([P, ROUNDS, KT * BC], mybir.dt.uint8, tag="carry")
            nc.sync.dma_start(carry_sb[:], carry.ap().rearrange("r p c -> p r c"))
            cinit_sb = cpool.tile([P, ROUNDS, KT * BC], BF16, tag="cinit")
            nc.sync.dma_start(cinit_sb[:], cinit.ap().rearrange("r p c -> p r c"))
            gidx_sb = cpool.tile([P, 1], I32, tag="gidx")
            nc.sync.dma_start(gidx_sb[:], gidx[:])
            id_sb = cpool.tile([P, P], BF16, tag="ident")
            nc.sync.dma_start(id_sb[:], ident[:])

            # persistent double buffers, alternated by round parity
            blkA = blkpool.tile([P, MT, T, BC], BF16, tag="blkA")
            blkB = blkpool.tile([P, MT, T, BC], BF16, tag="blkB")
            nc.vector.memset(blkA[:], 0.0)
            nc.vector.memset(blkB[:], 0.0)
            # gathered input block, cols (h, k, tl, b)
            xbA = blkpool.tile([P, 2, HALF_COLS], BF16, tag="xbA")
            xbB = blkpool.tile([P, 2, HALF_COLS], BF16, tag="xbB")

            xw_sb = xwpool.tile([P, MT, T, BC], BF16, tag="xw")

            def gather(xb, h, r, deps):
                g = nc.gpsimd.indirect_dma_start(
                    out=xb[:, h, :],
                    out_offset=None,
                    in_=hands[h][r][:],
                    in_offset=bass.IndirectOffsetOnAxis(ap=gidx_sb[:, :1], axis=0),
                )
                for dep, reason in deps:
                    add_dep_helper(g.ins, dep.ins, sync=True, reason=reason)
                return g

            def proj_pair(xb, h, mp, after=None):
                """One m-pair PSUM group of the projection for half h.

                The pair's matmuls are chained sequentially and anchored
                after `after` (sync=False scheduling edges) so the
                scheduler cannot float them to the head of the PE queue,
                where their gather dependency would head-of-line-block
                the recurrence.
                """
                pp = pppool.tile([P, 2, TH, 16], mybir.dt.float32, tag="pp")
                prev_mm = None
                for k in range(KT):
                    for mi in range(2):
                        m = 2 * mp + mi
                        mm = nc.tensor.matmul(
                            pp[:, mi, :, 0:BC],
                            wx_sb[:, k, m, :],
                            xb[:, h, k * TH * BC:(k + 1) * TH * BC],
                            start=(k == 0 and mi == 0),
                            stop=(k == KT - 1 and mi == 1),
                            skip_group_check=True,
                        )
                        if prev_mm is None:
                            if after is not None:
                                add_dep_helper(mm.ins, after.ins, sync=False,
                                               reason="hold proj in place")
                        else:
                            add_dep_helper(mm.ins, prev_mm.ins, sync=False,
                                           reason="keep pair contiguous")
                        prev_mm = mm
                nc.vector.tensor_tensor(
                    out=xw_sb[:, 2 * mp:2 * mp + 2, h * TH:(h + 1) * TH, :],
                    in0=pp[:, :, :, 0:BC],
                    in1=bias_sb[:, 2 * mp:2 * mp + 2, None, None].to_broadcast((P, 2, TH, BC)),
                    op=mybir.AluOpType.add,
                )

            def rec_step(cur, hstart, t):
                """Returns the step's last wh matmul (proj anchor)."""
                pslo = prpool.tile([P, 4, 128], mybir.dt.float32, tag="pslo")
                pshi = prpool.tile([P, 4, 128], mybir.dt.float32, tag="pshi")
                idlo = nc.tensor.matmul(
                    pslo[:, :, 0:BC], id_sb[:], xw_sb[:, 0:4, t, :],
                    start=True, stop=False, skip_group_check=True,
                )
                idhi = nc.tensor.matmul(
                    pshi[:, :, 0:BC], id_sb[:], xw_sb[:, 4:8, t, :],
                    start=True, stop=False, skip_group_check=True,
                )
                for half in range(2):
                    ps = pslo if half == 0 else pshi
                    idm = idlo if half == 0 else idhi
                    for k in range(KT):
                        if t == 0:
                            rhs = hstart[:, k * BC:(k + 1) * BC]
                        else:
                            rhs = cur[:, k, t - 1, :]
                        for mi in range(4):
                            m = 4 * half + mi
                            mm = nc.tensor.matmul(
                                ps[:, mi, 0:BC],
                                wh_sb[:, k, m, :],
                                rhs,
                                start=False,
                                stop=(k == KT - 1 and mi == 3),
                                skip_group_check=True,
                            )
                            if k == 0:
                                add_dep_helper(mm.ins, idm.ins, sync=False,
                                               reason="bank clear first")
                    nc.scalar.activation(
                        cur[:, 4 * half:4 * half + 4, t, :],
                        ps[:, :, 0:BC], Tanh
                    )
                return mm

            # stage round-0 x-feeds
            dx0 = nc.sync.dma_start(hands[0][0][4 * P:5 * P, :], x0t[0][:, 0:HALF_COLS])
            dx1 = nc.sync.dma_start(hands[1][0][4 * P:5 * P, :], x0t[0][:, HALF_COLS:])
            dx_prev = [dx0, dx1]

            cc_h0 = cc_h1 = None
            for r in range(ROUNDS):
                cur = blkA if r % 2 == 0 else blkB
                prev = blkB if r % 2 == 0 else blkA
                xb = xbA if r % 2 == 0 else xbB
                xb_next = xbB if r % 2 == 0 else xbA

                if r == 0:
                    # no prior round overlapped this work: gather+project
                    # the first half up front.
                    g0 = gather(xb, 0, 0, [(dx_prev[0], "gather after x-feed")])
                    for mp in range(4):
                        proj_pair(xb, 0, mp)

                # second-half gather for this round (its AG launched at the
                # end of round r-1; projection is interleaved into steps
                # 8..15 below).
                deps = [(dx_prev[1], "gather after x-feed")]
                if cc_h1 is not None:
                    deps.append((cc_h1, "gather after AG"))
                gather(xb, 1, r, deps)

                # h_start = carry ? prev_block_tail : cinit
                hstart = hspool.tile([P, KT * BC], BF16, tag="hs")
                nc.vector.tensor_copy(hstart[:], cinit_sb[:, r])
                nc.vector.copy_predicated(
                    hstart[:], carry_sb[:, r], prev[:, :, T - 1, :]
                )

                for t in range(T):
                    last_mm = rec_step(cur, hstart, t)

                    # interleave projections into the tanh-latency gaps:
                    # steps 8..11 project this round's second half (one
                    # m-pair per step), steps 24..27 the next round's
                    # first half (gathered below after its mid-round AG).
                    if 8 <= t < 12:
                        proj_pair(xb, 1, t - 8, after=last_mm)
                    if 24 <= t < 28 and r < ROUNDS - 1:
                        proj_pair(xb_next, 0, t - 24, after=last_mm)

                    if t == TH - 1 and r < ROUNDS - 1:
                        # ship the first half-block mid-round
                        d1 = nc.sync.dma_start(
                            hins[0][r].ap().rearrange("p (m t b) -> p m t b", m=MT, t=TH),
                            cur[:, :, 0:TH, :],
                        )
                        dxn0 = nc.sync.dma_start(
                            hands[0][r + 1][4 * P:5 * P, :],
                            x0t[r + 1][:, 0:HALF_COLS])
                        cc_h0 = nc.gpsimd.collective_compute(
                            "AllGather",
                            mybir.AluOpType.bypass,
                            replica_groups=GROUPS,
                            ins=[hins[0][r][:]],
                            outs=[hands[0][r + 1][0:4 * P, :]],
                        )
                        add_dep_helper(cc_h0.ins, d1.ins, sync=True,
                                       reason="AG after blk dma")
                        gather(xb_next, 0, r + 1,
                               [(cc_h0, "gather after AG"),
                                (dxn0, "gather after x-feed")])

                # ship the second half-block at round end, BEFORE the
                # fp32 output copy: the shared Sync DMA queue would
                # otherwise delay the AG launch behind the 1.2us cast.
                if r < ROUNDS - 1:
                    d2 = nc.sync.dma_start(
                        hins[1][r].ap().rearrange("p (m t b) -> p m t b", m=MT, t=TH),
                        cur[:, :, TH:T, :],
                    )
                    dxn1 = nc.sync.dma_start(
                        hands[1][r + 1][4 * P:5 * P, :],
                        x0t[r + 1][:, HALF_COLS:])
                    cc_h1 = nc.gpsimd.collective_compute(
                        "AllGather",
                        mybir.AluOpType.bypass,
                        replica_groups=GROUPS,
                        ins=[hins[1][r][:]],
                        outs=[hands[1][r + 1][0:4 * P, :]],
                    )
                    add_dep_helper(cc_h1.ins, d2.ins, sync=True,
                                   reason="AG after blk dma")
                    dx_prev = [dxn0, dxn1]

                # write fp32 output block
                o32 = opool.tile([P, MT * T * BC], F32, tag="o32")
                nc.vector.tensor_copy(o32[:], cur[:])
                nc.sync.dma_start(out[r], o32[:])
    nc.compile()
    return nc


def _prep_inputs(X, h0s, W, b):
    """Build the 8 per-core input maps."""
    in_maps = []
    for c in range(N_CORES):
        s, j = c // 2, c % 2
        Wl = np.asarray(W[s], dtype=np.float32)
        Wx, Wh = Wl[:, :D], Wl[:, D:]

        def tiles(M):  # M: [e, d] -> lhsT tiles [p, (k, m, q)]
            A = M.reshape(MT, P, KT, P)          # [m, q, k, p]
            return np.ascontiguousarray(
                A.transpose(3, 2, 0, 1).reshape(P, KT * MT * P)).astype(BF)

        whT = tiles(Wh)
        wxT = tiles(Wx)
        bias = np.ascontiguousarray(
            np.asarray(b[s], np.float32).reshape(MT, P).T)

        hin = np.asarray(h0s[s, BC * j:BC * (j + 1)], np.float32)  # [b, d]
        hinit = np.ascontiguousarray(
            hin.reshape(BC, KT, P).transpose(2, 1, 0).reshape(P, KT * BC)).astype(BF)

        carry = np.zeros((ROUNDS, P, KT * BC), np.uint8)
        cinit = np.zeros((ROUNDS, P, KT * BC), BF)
        for r in range(ROUNDS):
            if r > s:
                carry[r] = 1
            else:
                cinit[r] = hinit

        x0t = np.zeros((ROUNDS, P, BLK_COLS), BF)
        if s == 0:
            Xj = np.asarray(X[BC * j:BC * (j + 1)], np.float32)  # [b, L, d]
            # [b, blk, h, tl, k, p] -> [blk, p, h, k, tl, b]
            Xb = Xj.reshape(BC, NB, 2, TH, KT, P).transpose(1, 5, 2, 4, 3, 0)
            Xb = np.ascontiguousarray(Xb.reshape(NB, P, BLK_COLS)).astype(BF)
            x0t[0:NB] = Xb
            gidx = (4 * P + np.arange(P, dtype=np.int32)).reshape(P, 1)
        else:  # stage s reads its predecessor (group position s-1)
            gidx = ((s - 1) * P + np.arange(P, dtype=np.int32)).reshape(P, 1)

        in_maps.append({
            "whT": whT, "wxT": wxT, "bias": bias,
            "carry": carry, "cinit": cinit,
            "gidx": gidx, "ident": np.eye(P, dtype=BF),
            "x0t": x0t,
        })
    return in_maps


def _extract(results):
    """Assemble full output [B, L, D] from stage-3 cores (6, 7)."""
    Y = np.empty((B, L, D), np.float32)
    for j in range(2):
        o = results[6 + j]["out"][NL - 1:NL - 1 + NB]   # [q, p, cols]
        o = o.reshape(NB, P, MT, T, BC).transpose(4, 0, 3, 2, 1)  # [b,q,t,m,p]
        Y[BC * j:BC * (j + 1)] = o.reshape(BC, L, D)
    return Y


def kernel(X, h0s, W, b, _trace=False):
    from concourse.bass_utils import run_bass_kernel_spmd

    if "nc" not in _cache:
        _cache["nc"] = _build()
    nc = _cache["nc"]
    in_maps = _prep_inputs(np.asarray(X), np.asarray(h0s), np.asarray(W),
                           np.asarray(b))
    res = run_bass_kernel_spmd(nc, in_maps, core_ids=list(range(N_CORES)),
                               trace=_trace)
    _cache["last_results"] = res
    return _extract(res.results)


# revision 26
# speedup vs baseline: 1.6603x; 1.1009x over previous
"""4-layer tanh RNN on 8 Trainium2 NeuronCores.

Strategy: 4-stage layer pipeline x 2-way batch split. Core c handles
layer c//2 for batch half c%2. Time is processed in blocks of T=32
steps; each round every core gathers its input block (previous stage's
output), projects it (x @ WxT + b), runs 32 recurrence steps
(weight-stationary bf16 matmuls, zT[d_out,b] layout so h never needs a
transpose), then hands its output block to the next stage.

Pipelining (v4): the handoff is split into half-blocks. The first half
(t<16) ships mid-round, so its AllGather and the successor's gather +
projection all overlap the producer's second half; the second half
ships at round end and is projected by the consumer mid-next-round.
Projection matmuls are interleaved a few per recurrence step, riding
the PE idle gaps the per-step tanh dependency creates, so projection
costs almost no wall-clock. Stage handoff uses one 4-core-group
AllGather per half along each batch-half chain (group position = stage,
2x0.5MB out/core/round vs 8MB for a flat 8-way gather); stage-0 cores
read their x-feed from a board region staged by local DMA. One
SPMD-uniform indirect gather with a per-core constant row index reads
the board.

Recurrence step: two PSUM groups (m-halves) in bank-padded tiles; an
identity matmul seeds xw into each group (start=True clears the bank),
32 wh matmuls accumulate, tanh reads PSUM directly as soon as its
half's group stops.

Compute dtype bf16 (PE fp32 is 4x slower), fp32 PSUM accumulation, fp32
tanh. xw is stored bf16 (it feeds the identity matmul).
"""
import sys
import numpy as np

if "/opt/trn_rl_repo" not in sys.path:
    sys.path.insert(0, "/opt/trn_rl_repo")

import ml_dtypes

BF = ml_dtypes.bfloat16

# Problem config (hardcoded per contract)
import os as _os
B, L, D, NL = 16, 512, 1024, 4
P = 128
KT = D // P          # 8 k-tiles (contraction)
MT = D // P          # 8 m-tiles (output)
BC = B // 2          # 8 = per-core batch half
T = 32               # timesteps per block
TH = T // 2          # 16 = half-block timesteps
NB = L // T          # 16 blocks
ROUNDS = NB + NL - 1  # 19
N_CORES = 8
BLK_COLS = MT * T * BC   # 2048 block columns
HALF_COLS = MT * TH * BC  # 1024 half-block columns: col = m*TH*BC + tl*BC + b

# one chain per batch half: stage s core is group position s, so each
# receiver reads its predecessor's block at rows (s-1)*P of the AG out.
GROUPS = [[0, 2, 4, 6], [1, 3, 5, 7]]

_cache = {}


def _build():
    import concourse.bass as bass
    import concourse.mybir as mybir
    import concourse.tile as tile
    from concourse import bacc
    from concourse.tile import add_dep_helper

    F32 = mybir.dt.float32
    BF16 = mybir.dt.bfloat16
    I32 = mybir.dt.int32
    Tanh = mybir.ActivationFunctionType.Tanh

    nc = bacc.Bacc("TRN2", target_bir_lowering=False, debug=False,
                   num_devices=N_CORES)

    # ---- I/O ----
    whT = nc.dram_tensor("whT", [P, KT * MT * P], BF16, kind="ExternalInput")
    wxT = nc.dram_tensor("wxT", [P, KT * MT * P], BF16, kind="ExternalInput")
    bias = nc.dram_tensor("bias", [P, MT], F32, kind="ExternalInput")
    carry = nc.dram_tensor("carry", [ROUNDS, P, KT * BC], mybir.dt.uint8, kind="ExternalInput")
    cinit = nc.dram_tensor("cinit", [ROUNDS, P, KT * BC], BF16, kind="ExternalInput")
    gidx = nc.dram_tensor("gidx", [P, 1], I32, kind="ExternalInput")
    ident = nc.dram_tensor("ident", [P, P], BF16, kind="ExternalInput")
    # x-feed, halved column layout: col = h*HALF_COLS + k*TH*BC + tl*BC + b
    x0t = nc.dram_tensor("x0t", [ROUNDS, P, BLK_COLS], BF16, kind="ExternalInput")
    out = nc.dram_tensor("out", [ROUNDS, P, BLK_COLS], F32, kind="ExternalOutput")

    # handoff boards, one per half-block per round: rows [0:4P) AllGather
    # out (one row-block per chain stage), [4P:5P) x-feed staged by DMA.
    hands = [[nc.dram_tensor(f"hand{h}_{r}", [5 * P, HALF_COLS], BF16)
              for r in range(ROUNDS)] for h in range(2)]
    hins = [[nc.dram_tensor(f"hin{h}_{r}", [P, HALF_COLS], BF16)
             for r in range(ROUNDS - 1)] for h in range(2)]

    with tile.TileContext(nc) as tc:
        with (
            tc.tile_pool(name="const", bufs=1) as cpool,
            tc.tile_pool(name="xw", bufs=1) as xwpool,
            tc.tile_pool(name="blk", bufs=1) as blkpool,
            tc.tile_pool(name="hs", bufs=2) as hspool,
            tc.tile_pool(name="o32", bufs=2) as opool,
            tc.tile_pool(name="psr", bufs=2, space="PSUM") as prpool,
            tc.tile_pool(name="psp", bufs=2, space="PSUM") as pppool,
        ):
            wh_sb = cpool.tile([P, KT, MT, P], BF16, tag="wh")
            nc.sync.dma_start(wh_sb[:], whT.ap().rearrange("p (k m q) -> p k m q", k=KT, m=MT))
            wx_sb = cpool.tile([P, KT, MT, P], BF16, tag="wx")
            nc.sync.dma_start(wx_sb[:], wxT.ap().rearrange("p (k m q) -> p k m q", k=KT, m=MT))
            bias_sb = cpool.tile([P, MT], F32, tag="bias")
            nc.sync.dma_start(bias_sb[:], bias[:])
            carry_sb = cpool.tile([P, ROUNDS, KT * BC], mybir.dt.uint8, tag="carry")
            nc.sync.dma_start(carry_sb[:], carry.ap().rearrange("r p c -> p r c"))
            cinit_sb = cpool.tile([P, ROUNDS, KT * BC], BF16, tag="cinit")
            nc.sync.dma_start(cinit_sb[:], cinit.ap().rearrange("r p c -> p r c"))
            gidx_sb = cpool.tile([P, 1], I32, tag="gidx")
            nc.sync.dma_start(gidx_sb[:], gidx[:])
            id_sb = cpool.tile([P, P], BF16, tag="ident")
            nc.sync.dma_start(id_sb[:], ident[:])

            # persistent double buffers, alternated by round parity
            blkA = blkpool.tile([P, MT, T, BC], BF16, tag="blkA")
            blkB = blkpool.tile([P, MT, T, BC], BF16, tag="blkB")
            nc.vector.memset(blkA[:], 0.0)
            nc.vector.memset(blkB[:], 0.0)
            # gathered input block, cols (h, k, tl, b)
            xbA = blkpool.tile([P, 2, HALF_COLS], BF16, tag="xbA")
            xbB = blkpool.tile([P, 2, HALF_COLS], BF16, tag="xbB")

            xw_sb = xwpool.tile([P, MT, T, BC], BF16, tag="xw")

            def gather(xb, h, r, deps):
                g = nc.gpsimd.indirect_dma_start(
                    out=xb[:, h, :],
                    out_offset=None,
                    in_=hands[h][r][:],
                    in_offset=bass.IndirectOffsetOnAxis(ap=gidx_sb[:, :1], axis=0),
                )
                for dep, reason in deps:
                    add_dep_helper(g.ins, dep.ins, sync=True, reason=reason)
                return g

            def proj_chunk(pp, xb, h, mp, kr, after=None):
                """Half of an m-pair projection group (4 k-tiles x 2 m).

                Matmuls are chained sequentially and anchored after
                `after` (sync=False scheduling edges) so the scheduler
                cannot float them to the head of the PE queue, where
                their gather dependency would head-of-line-block the
                recurrence.
                """
                prev_mm = None
                for k in range(4 * kr, 4 * kr + 4):
                    for mi in range(2):
                        m = 2 * mp + mi
                        mm = nc.tensor.matmul(
                            pp[:, mi, :, 0:BC],
                            wx_sb[:, k, m, :],
                            xb[:, h, k * TH * BC:(k + 1) * TH * BC],
                            start=(k == 0 and mi == 0),
                            stop=(k == KT - 1 and mi == 1),
                            skip_group_check=True,
                        )
                        if prev_mm is None:
                            if after is not None:
                                add_dep_helper(mm.ins, after.ins, sync=False,
                                               reason="hold proj in place")
                        else:
                            add_dep_helper(mm.ins, prev_mm.ins, sync=False,
                                           reason="keep chunk contiguous")
                        prev_mm = mm
                if kr == 1:
                    nc.vector.tensor_tensor(
                        out=xw_sb[:, 2 * mp:2 * mp + 2, h * TH:(h + 1) * TH, :],
                        in0=pp[:, :, :, 0:BC],
                        in1=bias_sb[:, 2 * mp:2 * mp + 2, None, None].to_broadcast((P, 2, TH, BC)),
                        op=mybir.AluOpType.add,
                    )

            def proj_pair(xb, h, mp, after=None):
                pp = pppool.tile([P, 2, TH, 16], mybir.dt.float32, tag="pp")
                proj_chunk(pp, xb, h, mp, 0, after)
                proj_chunk(pp, xb, h, mp, 1)

            def rec_step(cur, hstart, t):
                """Returns the step's last wh matmul (proj anchor)."""
                pslo = prpool.tile([P, 4, 128], mybir.dt.float32, tag="pslo")
                pshi = prpool.tile([P, 4, 128], mybir.dt.float32, tag="pshi")
                idlo = nc.tensor.matmul(
                    pslo[:, :, 0:BC], id_sb[:], xw_sb[:, 0:4, t, :],
                    start=True, stop=False, skip_group_check=True,
                )
                idhi = nc.tensor.matmul(
                    pshi[:, :, 0:BC], id_sb[:], xw_sb[:, 4:8, t, :],
                    start=True, stop=False, skip_group_check=True,
                )
                for half in range(2):
                    ps = pslo if half == 0 else pshi
                    idm = idlo if half == 0 else idhi
                    for k in range(KT):
                        if t == 0:
                            rhs = hstart[:, k * BC:(k + 1) * BC]
                        else:
                            rhs = cur[:, k, t - 1, :]
                        for mi in range(4):
                            m = 4 * half + mi
                            mm = nc.tensor.matmul(
                                ps[:, mi, 0:BC],
                                wh_sb[:, k, m, :],
                                rhs,
                                start=False,
                                stop=(k == KT - 1 and mi == 3),
                                skip_group_check=True,
                            )
                            if k == 0:
                                add_dep_helper(mm.ins, idm.ins, sync=False,
                                               reason="bank clear first")
                    nc.scalar.activation(
                        cur[:, 4 * half:4 * half + 4, t, :],
                        ps[:, :, 0:BC], Tanh
                    )
                return mm

            # stage round-0 x-feeds
            dx0 = nc.sync.dma_start(hands[0][0][4 * P:5 * P, :], x0t[0][:, 0:HALF_COLS])
            dx1 = nc.sync.dma_start(hands[1][0][4 * P:5 * P, :], x0t[0][:, HALF_COLS:])
            dx_prev = [dx0, dx1]

            cc_h0 = cc_h1 = None
            for r in range(ROUNDS):
                cur = blkA if r % 2 == 0 else blkB
                prev = blkB if r % 2 == 0 else blkA
                xb = xbA if r % 2 == 0 else xbB
                xb_next = xbB if r % 2 == 0 else xbA

                if r == 0:
                    # no prior round overlapped this work: gather+project
                    # the first half up front.
                    g0 = gather(xb, 0, 0, [(dx_prev[0], "gather after x-feed")])
                    for mp in range(4):
                        proj_pair(xb, 0, mp)

                # second-half gather for this round (its AG launched at the
                # end of round r-1; projection is interleaved into steps
                # 8..15 below).
                deps = [(dx_prev[1], "gather after x-feed")]
                if cc_h1 is not None:
                    deps.append((cc_h1, "gather after AG"))
                g_h1 = gather(xb, 1, r, deps)

                # h_start = carry ? prev_block_tail : cinit
                hstart = hspool.tile([P, KT * BC], BF16, tag="hs")
                nc.vector.tensor_copy(hstart[:], cinit_sb[:, r])
                nc.vector.copy_predicated(
                    hstart[:], carry_sb[:, r], prev[:, :, T - 1, :]
                )

                pp_live = None
                for t in range(T):
                    last_mm = rec_step(cur, hstart, t)

                    # interleave projections into the tanh-latency gaps,
                    # 8 matmuls (half an m-pair group) per step: steps
                    # 8..15 project this round's second half, steps
                    # 24..31 the next round's first half (gathered below
                    # after its mid-round AG).
                    if 8 <= t < 16:
                        mp, kr = divmod(t - 8, 2)
                        if kr == 0:
                            pp_live = pppool.tile([P, 2, TH, 16],
                                                  mybir.dt.float32, tag="pp")
                        proj_chunk(pp_live, xb, 1, mp, kr, after=last_mm)
                    if 24 <= t < 32 and r < ROUNDS - 1:
                        mp, kr = divmod(t - 24, 2)
                        if kr == 0:
                            pp_live = pppool.tile([P, 2, TH, 16],
                                                  mybir.dt.float32, tag="pp")
                        proj_chunk(pp_live, xb_next, 0, mp, kr, after=last_mm)

                    if t == TH - 1 and r < ROUNDS - 1:
                        # ship the first half-block mid-round
                        d1 = nc.sync.dma_start(
                            hins[0][r].ap().rearrange("p (m t b) -> p m t b", m=MT, t=TH),
                            cur[:, :, 0:TH, :],
                        )
                        dxn0 = nc.sync.dma_start(
                            hands[0][r + 1][4 * P:5 * P, :],
                            x0t[r + 1][:, 0:HALF_COLS])
                        cc_h0 = nc.gpsimd.collective_compute(
                            "AllGather",
                            mybir.AluOpType.bypass,
                            replica_groups=GROUPS,
                            ins=[hins[0][r][:]],
                            outs=[hands[0][r + 1][0:4 * P, :]],
                        )
                        add_dep_helper(cc_h0.ins, d1.ins, sync=True,
                                       reason="AG after blk dma")
                        # keep the gpsimd queue in program order: the
                        # scheduler must not move this AG trigger ahead
                        # of the round-top gather (whose wait would then
                        # head-of-line-block it).
                        add_dep_helper(cc_h0.ins, g_h1.ins, sync=False,
                                       reason="queue order")
                        g_h0n = gather(xb_next, 0, r + 1,
                                       [(cc_h0, "gather after AG"),
                                        (dxn0, "gather after x-feed")])

                # ship the second half-block at round end, BEFORE the
                # fp32 output copy: the shared Sync DMA queue would
                # otherwise delay the AG launch behind the 1.2us cast.
                if r < ROUNDS - 1:
                    d2 = nc.sync.dma_start(
                        hins[1][r].ap().rearrange("p (m t b) -> p m t b", m=MT, t=TH),
                        cur[:, :, TH:T, :],
                    )
                    dxn1 = nc.sync.dma_start(
                        hands[1][r + 1][4 * P:5 * P, :],
                        x0t[r + 1][:, HALF_COLS:])
                    cc_h1 = nc.gpsimd.collective_compute(
                        "AllGather",
                        mybir.AluOpType.bypass,
                        replica_groups=GROUPS,
                        ins=[hins[1][r][:]],
                        outs=[hands[1][r + 1][0:4 * P, :]],
                    )
                    add_dep_helper(cc_h1.ins, d2.ins, sync=True,
                                   reason="AG after blk dma")
                    # the next-round h0 gather must run as soon as its AG
                    # lands, not behind this trigger's step-31 DMA wait.
                    add_dep_helper(cc_h1.ins, g_h0n.ins, sync=False,
                                   reason="queue order")
                    dx_prev = [dxn0, dxn1]

                # write fp32 output block
                o32 = opool.tile([P, MT * T * BC], F32, tag="o32")
                nc.vector.tensor_copy(o32[:], cur[:])
                nc.sync.dma_start(out[r], o32[:])
    nc.compile()
    return nc


def _prep_inputs(X, h0s, W, b):
    """Build the 8 per-core input maps."""
    in_maps = []
    for c in range(N_CORES):
        s, j = c // 2, c % 2
        Wl = np.asarray(W[s], dtype=np.float32)
        Wx, Wh = Wl[:, :D], Wl[:, D:]

        def tiles(M):  # M: [e, d] -> lhsT tiles [p, (k, m, q)]
            A = M.reshape(MT, P, KT, P)          # [m, q, k, p]
            return np.ascontiguousarray(
                A.transpose(3, 2, 0, 1).reshape(P, KT * MT * P)).astype(BF)

        whT = tiles(Wh)
        wxT = tiles(Wx)
        bias = np.ascontiguousarray(
            np.asarray(b[s], np.float32).reshape(MT, P).T)

        hin = np.asarray(h0s[s, BC * j:BC * (j + 1)], np.float32)  # [b, d]
        hinit = np.ascontiguousarray(
            hin.reshape(BC, KT, P).transpose(2, 1, 0).reshape(P, KT * BC)).astype(BF)

        carry = np.zeros((ROUNDS, P, KT * BC), np.uint8)
        cinit = np.zeros((ROUNDS, P, KT * BC), BF)
        for r in range(ROUNDS):
            if r > s:
                carry[r] = 1
            else:
                cinit[r] = hinit

        x0t = np.zeros((ROUNDS, P, BLK_COLS), BF)
        if s == 0:
            Xj = np.asarray(X[BC * j:BC * (j + 1)], np.float32)  # [b, L, d]
            # [b, blk, h, tl, k, p] -> [blk, p, h, k, tl, b]
            Xb = Xj.reshape(BC, NB, 2, TH, KT, P).transpose(1, 5, 2, 4, 3, 0)
            Xb = np.ascontiguousarray(Xb.reshape(NB, P, BLK_COLS)).astype(BF)
            x0t[0:NB] = Xb
            gidx = (4 * P + np.arange(P, dtype=np.int32)).reshape(P, 1)
        else:  # stage s reads its predecessor (group position s-1)
            gidx = ((s - 1) * P + np.arange(P, dtype=np.int32)).reshape(P, 1)

        in_maps.append({
            "whT": whT, "wxT": wxT, "bias": bias,
            "carry": carry, "cinit": cinit,
            "gidx": gidx, "ident": np.eye(P, dtype=BF),
            "x0t": x0t,
        })
    return in_maps


def _extract(results):
    """Assemble full output [B, L, D] from stage-3 cores (6, 7)."""
    Y = np.empty((B, L, D), np.float32)
    for j in range(2):
        o = results[6 + j]["out"][NL - 1:NL - 1 + NB]   # [q, p, cols]
        o = o.reshape(NB, P, MT, T, BC).transpose(4, 0, 3, 2, 1)  # [b,q,t,m,p]
        Y[BC * j:BC * (j + 1)] = o.reshape(BC, L, D)
    return Y


def kernel(X, h0s, W, b, _trace=False):
    from concourse.bass_utils import run_bass_kernel_spmd

    if "nc" not in _cache:
        _cache["nc"] = _build()
    nc = _cache["nc"]
    in_maps = _prep_inputs(np.asarray(X), np.asarray(h0s), np.asarray(W),
                           np.asarray(b))
    res = run_bass_kernel_spmd(nc, in_maps, core_ids=list(range(N_CORES)),
                               trace=_trace)
    _cache["last_results"] = res
    return _extract(res.results)


# revision 32
# speedup vs baseline: 1.6630x; 1.0016x over previous
"""4-layer tanh RNN on 8 Trainium2 NeuronCores.

Strategy: 4-stage layer pipeline x 2-way batch split. Core c handles
layer c//2 for batch half c%2. Time is processed in blocks of T=32
steps; each round every core gathers its input block (previous stage's
output), projects it (x @ WxT + b), runs 32 recurrence steps
(weight-stationary bf16 matmuls, zT[d_out,b] layout so h never needs a
transpose), then hands its output block to the next stage.

Pipelining (v4): the handoff is split into half-blocks. The first half
(t<16) ships mid-round, so its AllGather and the successor's gather +
projection all overlap the producer's second half; the second half
ships at round end and is projected by the consumer mid-next-round.
Projection matmuls are interleaved a few per recurrence step, riding
the PE idle gaps the per-step tanh dependency creates, so projection
costs almost no wall-clock. Stage handoff uses one 4-core-group
AllGather per half along each batch-half chain (group position = stage,
2x0.5MB out/core/round vs 8MB for a flat 8-way gather); stage-0 cores
read their x-feed from a board region staged by local DMA. One
SPMD-uniform indirect gather with a per-core constant row index reads
the board.

Recurrence step: two PSUM groups (m-halves) in bank-padded tiles; an
identity matmul seeds xw into each group (start=True clears the bank),
32 wh matmuls accumulate, tanh reads PSUM directly as soon as its
half's group stops.

Compute dtype bf16 (PE fp32 is 4x slower), fp32 PSUM accumulation, fp32
tanh. xw is stored bf16 (it feeds the identity matmul).
"""
import sys
import numpy as np

if "/opt/trn_rl_repo" not in sys.path:
    sys.path.insert(0, "/opt/trn_rl_repo")

import ml_dtypes

BF = ml_dtypes.bfloat16

# Problem config (hardcoded per contract)
import os as _os
B, L, D, NL = 16, 512, 1024, 4
P = 128
KT = D // P          # 8 k-tiles (contraction)
MT = D // P          # 8 m-tiles (output)
BC = B // 2          # 8 = per-core batch half
T = 32               # timesteps per block
TH = T // 2          # 16 = half-block timesteps
NB = L // T          # 16 blocks
ROUNDS = NB + NL - 1  # 19
N_CORES = 8
BLK_COLS = MT * T * BC   # 2048 block columns
HALF_COLS = MT * TH * BC  # 1024 half-block columns: col = m*TH*BC + tl*BC + b

# one chain per batch half: stage s core is group position s, so each
# receiver reads its predecessor's block at rows (s-1)*P of the AG out.
GROUPS = [[0, 2, 4, 6], [1, 3, 5, 7]]

_cache = {}


def _build():
    import concourse.bass as bass
    import concourse.mybir as mybir
    import concourse.tile as tile
    from concourse import bacc
    from concourse.tile import add_dep_helper

    F32 = mybir.dt.float32
    BF16 = mybir.dt.bfloat16
    I32 = mybir.dt.int32
    Tanh = mybir.ActivationFunctionType.Tanh

    nc = bacc.Bacc("TRN2", target_bir_lowering=False, debug=False,
                   num_devices=N_CORES)

    # ---- I/O ----
    whT = nc.dram_tensor("whT", [P, KT * MT * P], BF16, kind="ExternalInput")
    wxT = nc.dram_tensor("wxT", [P, KT * MT * P], BF16, kind="ExternalInput")
    bias = nc.dram_tensor("bias", [P, MT], F32, kind="ExternalInput")
    carry = nc.dram_tensor("carry", [ROUNDS, P, KT * BC], mybir.dt.uint8, kind="ExternalInput")
    cinit = nc.dram_tensor("cinit", [ROUNDS, P, KT * BC], BF16, kind="ExternalInput")
    gidx = nc.dram_tensor("gidx", [P, 1], I32, kind="ExternalInput")
    ident = nc.dram_tensor("ident", [P, P], BF16, kind="ExternalInput")
    # x-feed, halved column layout: col = h*HALF_COLS + k*TH*BC + tl*BC + b
    x0t = nc.dram_tensor("x0t", [ROUNDS, P, BLK_COLS], BF16, kind="ExternalInput")
    out = nc.dram_tensor("out", [ROUNDS, P, BLK_COLS], F32, kind="ExternalOutput")

    # handoff boards, one per half-block per round: rows [0:4P) AllGather
    # out (one row-block per chain stage), [4P:5P) x-feed staged by DMA.
    hands = [[nc.dram_tensor(f"hand{h}_{r}", [5 * P, HALF_COLS], BF16)
              for r in range(ROUNDS)] for h in range(2)]
    hins = [[nc.dram_tensor(f"hin{h}_{r}", [P, HALF_COLS], BF16)
             for r in range(ROUNDS - 1)] for h in range(2)]
    wu_in = nc.dram_tensor("wu_in", [P, 64], BF16)
    wu_out = nc.dram_tensor("wu_out", [4 * P, 64], BF16)

    with tile.TileContext(nc) as tc:
        with (
            tc.tile_pool(name="const", bufs=1) as cpool,
            tc.tile_pool(name="xw", bufs=1) as xwpool,
            tc.tile_pool(name="blk", bufs=1) as blkpool,
            tc.tile_pool(name="hs", bufs=2) as hspool,
            tc.tile_pool(name="o32", bufs=2) as opool,
            tc.tile_pool(name="psr", bufs=2, space="PSUM") as prpool,
            tc.tile_pool(name="psp", bufs=2, space="PSUM") as pppool,
        ):
            wh_sb = cpool.tile([P, KT, MT, P], BF16, tag="wh")
            nc.sync.dma_start(wh_sb[:], whT.ap().rearrange("p (k m q) -> p k m q", k=KT, m=MT))
            wx_sb = cpool.tile([P, KT, MT, P], BF16, tag="wx")
            nc.sync.dma_start(wx_sb[:], wxT.ap().rearrange("p (k m q) -> p k m q", k=KT, m=MT))
            bias_sb = cpool.tile([P, MT], F32, tag="bias")
            nc.sync.dma_start(bias_sb[:], bias[:])
            carry_sb = cpool.tile([P, ROUNDS, KT * BC], mybir.dt.uint8, tag="carry")
            nc.sync.dma_start(carry_sb[:], carry.ap().rearrange("r p c -> p r c"))
            cinit_sb = cpool.tile([P, ROUNDS, KT * BC], BF16, tag="cinit")
            nc.sync.dma_start(cinit_sb[:], cinit.ap().rearrange("r p c -> p r c"))
            gidx_sb = cpool.tile([P, 1], I32, tag="gidx")
            nc.sync.dma_start(gidx_sb[:], gidx[:])
            id_sb = cpool.tile([P, P], BF16, tag="ident")
            nc.sync.dma_start(id_sb[:], ident[:])

            # persistent double buffers, alternated by round parity
            blkA = blkpool.tile([P, MT, T, BC], BF16, tag="blkA")
            blkB = blkpool.tile([P, MT, T, BC], BF16, tag="blkB")
            nc.vector.memset(blkA[:], 0.0)
            nc.vector.memset(blkB[:], 0.0)
            # gathered input block, cols (h, k, tl, b)
            xbA = blkpool.tile([P, 2, HALF_COLS], BF16, tag="xbA")
            xbB = blkpool.tile([P, 2, HALF_COLS], BF16, tag="xbB")

            xw_sb = xwpool.tile([P, MT, T, BC], BF16, tag="xw")

            def gather(xb, h, r, deps):
                g = nc.gpsimd.indirect_dma_start(
                    out=xb[:, h, :],
                    out_offset=None,
                    in_=hands[h][r][:],
                    in_offset=bass.IndirectOffsetOnAxis(ap=gidx_sb[:, :1], axis=0),
                )
                for dep, reason in deps:
                    add_dep_helper(g.ins, dep.ins, sync=True, reason=reason)
                return g

            def proj_chunk(pp, xb, h, mp, kr, after=None):
                """Half of an m-pair projection group (4 k-tiles x 2 m).

                Matmuls are chained sequentially and anchored after
                `after` (sync=False scheduling edges) so the scheduler
                cannot float them to the head of the PE queue, where
                their gather dependency would head-of-line-block the
                recurrence.
                """
                prev_mm = None
                for k in range(4 * kr, 4 * kr + 4):
                    for mi in range(2):
                        m = 2 * mp + mi
                        mm = nc.tensor.matmul(
                            pp[:, mi, :, 0:BC],
                            wx_sb[:, k, m, :],
                            xb[:, h, k * TH * BC:(k + 1) * TH * BC],
                            start=(k == 0 and mi == 0),
                            stop=(k == KT - 1 and mi == 1),
                            skip_group_check=True,
                        )
                        if prev_mm is None:
                            if after is not None:
                                add_dep_helper(mm.ins, after.ins, sync=False,
                                               reason="hold proj in place")
                        else:
                            add_dep_helper(mm.ins, prev_mm.ins, sync=False,
                                           reason="keep chunk contiguous")
                        prev_mm = mm
                if kr == 1:
                    nc.vector.tensor_tensor(
                        out=xw_sb[:, 2 * mp:2 * mp + 2, h * TH:(h + 1) * TH, :],
                        in0=pp[:, :, :, 0:BC],
                        in1=bias_sb[:, 2 * mp:2 * mp + 2, None, None].to_broadcast((P, 2, TH, BC)),
                        op=mybir.AluOpType.add,
                    )

            def proj_pair(xb, h, mp, after=None):
                pp = pppool.tile([P, 2, TH, 16], mybir.dt.float32, tag="pp")
                proj_chunk(pp, xb, h, mp, 0, after)
                proj_chunk(pp, xb, h, mp, 1)

            def rec_step(cur, hstart, t):
                """Returns the step's last wh matmul (proj anchor)."""
                pslo = prpool.tile([P, 4, 128], mybir.dt.float32, tag="pslo")
                pshi = prpool.tile([P, 4, 128], mybir.dt.float32, tag="pshi")
                idlo = nc.tensor.matmul(
                    pslo[:, :, 0:BC], id_sb[:], xw_sb[:, 0:4, t, :],
                    start=True, stop=False, skip_group_check=True,
                )
                idhi = nc.tensor.matmul(
                    pshi[:, :, 0:BC], id_sb[:], xw_sb[:, 4:8, t, :],
                    start=True, stop=False, skip_group_check=True,
                )
                for half in range(2):
                    ps = pslo if half == 0 else pshi
                    idm = idlo if half == 0 else idhi
                    for k in range(KT):
                        if t == 0:
                            rhs = hstart[:, k * BC:(k + 1) * BC]
                        else:
                            rhs = cur[:, k, t - 1, :]
                        for mi in range(4):
                            m = 4 * half + mi
                            mm = nc.tensor.matmul(
                                ps[:, mi, 0:BC],
                                wh_sb[:, k, m, :],
                                rhs,
                                start=False,
                                stop=(k == KT - 1 and mi == 3),
                                skip_group_check=True,
                            )
                            if k == 0:
                                add_dep_helper(mm.ins, idm.ins, sync=False,
                                               reason="bank clear first")
                    nc.scalar.activation(
                        cur[:, 4 * half:4 * half + 4, t, :],
                        ps[:, :, 0:BC], Tanh
                    )
                return mm

            # warm up the collective ring before the first real handoff
            # (the first AllGather pays ~25us of one-time setup)
            nc.gpsimd.collective_compute(
                "AllGather",
                mybir.AluOpType.bypass,
                replica_groups=GROUPS,
                ins=[wu_in[:]],
                outs=[wu_out[:]],
            )

            # stage round-0 x-feeds
            dx0 = nc.sync.dma_start(hands[0][0][4 * P:5 * P, :], x0t[0][:, 0:HALF_COLS])
            dx1 = nc.sync.dma_start(hands[1][0][4 * P:5 * P, :], x0t[0][:, HALF_COLS:])
            dx_prev = [dx0, dx1]

            cc_h0 = cc_h1 = None
            for r in range(ROUNDS):
                cur = blkA if r % 2 == 0 else blkB
                prev = blkB if r % 2 == 0 else blkA
                xb = xbA if r % 2 == 0 else xbB
                xb_next = xbB if r % 2 == 0 else xbA

                if r == 0:
                    # no prior round overlapped this work: gather+project
                    # the first half up front.
                    g0 = gather(xb, 0, 0, [(dx_prev[0], "gather after x-feed")])
                    for mp in range(4):
                        proj_pair(xb, 0, mp)

                # second-half gather for this round (its AG launched at the
                # end of round r-1; projection is interleaved into steps
                # 8..15 below).
                deps = [(dx_prev[1], "gather after x-feed")]
                if cc_h1 is not None:
                    deps.append((cc_h1, "gather after AG"))
                g_h1 = gather(xb, 1, r, deps)

                # h_start = carry ? prev_block_tail : cinit
                hstart = hspool.tile([P, KT * BC], BF16, tag="hs")
                nc.vector.tensor_copy(hstart[:], cinit_sb[:, r])
                nc.vector.copy_predicated(
                    hstart[:], carry_sb[:, r], prev[:, :, T - 1, :]
                )

                pp_live = None
                for t in range(T):
                    last_mm = rec_step(cur, hstart, t)

                    # interleave projections into the tanh-latency gaps,
                    # 8 matmuls (half an m-pair group) per step: steps
                    # 8..15 project this round's second half, steps
                    # 24..31 the next round's first half (gathered below
                    # after its mid-round AG).
                    if 8 <= t < 16:
                        mp, kr = divmod(t - 8, 2)
                        if kr == 0:
                            pp_live = pppool.tile([P, 2, TH, 16],
                                                  mybir.dt.float32, tag="pp")
                        proj_chunk(pp_live, xb, 1, mp, kr, after=last_mm)
                    if 24 <= t < 32 and r < ROUNDS - 1:
                        mp, kr = divmod(t - 24, 2)
                        if kr == 0:
                            pp_live = pppool.tile([P, 2, TH, 16],
                                                  mybir.dt.float32, tag="pp")
                        proj_chunk(pp_live, xb_next, 0, mp, kr, after=last_mm)

                    if t == TH - 1 and r < ROUNDS - 1:
                        # ship the first half-block mid-round
                        d1 = nc.sync.dma_start(
                            hins[0][r].ap().rearrange("p (m t b) -> p m t b", m=MT, t=TH),
                            cur[:, :, 0:TH, :],
                        )
                        dxn0 = nc.sync.dma_start(
                            hands[0][r + 1][4 * P:5 * P, :],
                            x0t[r + 1][:, 0:HALF_COLS])
                        cc_h0 = nc.gpsimd.collective_compute(
                            "AllGather",
                            mybir.AluOpType.bypass,
                            replica_groups=GROUPS,
                            ins=[hins[0][r][:]],
                            outs=[hands[0][r + 1][0:4 * P, :]],
                        )
                        add_dep_helper(cc_h0.ins, d1.ins, sync=True,
                                       reason="AG after blk dma")
                        # keep the gpsimd queue in program order: the
                        # scheduler must not move this AG trigger ahead
                        # of the round-top gather (whose wait would then
                        # head-of-line-block it).
                        add_dep_helper(cc_h0.ins, g_h1.ins, sync=False,
                                       reason="queue order")
                        g_h0n = gather(xb_next, 0, r + 1,
                                       [(cc_h0, "gather after AG"),
                                        (dxn0, "gather after x-feed")])

                # ship the second half-block at round end, BEFORE the
                # fp32 output copy: the shared Sync DMA queue would
                # otherwise delay the AG launch behind the 1.2us cast.
                if r < ROUNDS - 1:
                    d2 = nc.sync.dma_start(
                        hins[1][r].ap().rearrange("p (m t b) -> p m t b", m=MT, t=TH),
                        cur[:, :, TH:T, :],
                    )
                    dxn1 = nc.sync.dma_start(
                        hands[1][r + 1][4 * P:5 * P, :],
                        x0t[r + 1][:, HALF_COLS:])
                    cc_h1 = nc.gpsimd.collective_compute(
                        "AllGather",
                        mybir.AluOpType.bypass,
                        replica_groups=GROUPS,
                        ins=[hins[1][r][:]],
                        outs=[hands[1][r + 1][0:4 * P, :]],
                    )
                    add_dep_helper(cc_h1.ins, d2.ins, sync=True,
                                   reason="AG after blk dma")
                    # the next-round h0 gather must run as soon as its AG
                    # lands, not behind this trigger's step-31 DMA wait.
                    add_dep_helper(cc_h1.ins, g_h0n.ins, sync=False,
                                   reason="queue order")
                    dx_prev = [dxn0, dxn1]

                # write fp32 output block
                o32 = opool.tile([P, MT * T * BC], F32, tag="o32")
                nc.vector.tensor_copy(o32[:], cur[:])
                nc.sync.dma_start(out[r], o32[:])
    nc.compile()
    return nc


def _prep_inputs(X, h0s, W, b):
    """Build the 8 per-core input maps."""
    in_maps = []
    for c in range(N_CORES):
        s, j = c // 2, c % 2
        Wl = np.asarray(W[s], dtype=np.float32)
        Wx, Wh = Wl[:, :D], Wl[:, D:]

        def tiles(M):  # M: [e, d] -> lhsT tiles [p, (k, m, q)]
            A = M.reshape(MT, P, KT, P)          # [m, q, k, p]
            return np.ascontiguousarray(
                A.transpose(3, 2, 0, 1).reshape(P, KT * MT * P)).astype(BF)

        whT = tiles(Wh)
        wxT = tiles(Wx)
        bias = np.ascontiguousarray(
            np.asarray(b[s], np.float32).reshape(MT, P).T)

        hin = np.asarray(h0s[s, BC * j:BC * (j + 1)], np.float32)  # [b, d]
        hinit = np.ascontiguousarray(
            hin.reshape(BC, KT, P).transpose(2, 1, 0).reshape(P, KT * BC)).astype(BF)

        carry = np.zeros((ROUNDS, P, KT * BC), np.uint8)
        cinit = np.zeros((ROUNDS, P, KT * BC), BF)
        for r in range(ROUNDS):
            if r > s:
                carry[r] = 1
            else:
                cinit[r] = hinit

        x0t = np.zeros((ROUNDS, P, BLK_COLS), BF)
        if s == 0:
            Xj = np.asarray(X[BC * j:BC * (j + 1)], np.float32)  # [b, L, d]
            # [b, blk, h, tl, k, p] -> [blk, p, h, k, tl, b]
            Xb = Xj.reshape(BC, NB, 2, TH, KT, P).transpose(1, 5, 2, 4, 3, 0)
            Xb = np.ascontiguousarray(Xb.reshape(NB, P, BLK_COLS)).astype(BF)
            x0t[0:NB] = Xb
            gidx = (4 * P + np.arange(P, dtype=np.int32)).reshape(P, 1)
        else:  # stage s reads its predecessor (group position s-1)
            gidx = ((s - 1) * P + np.arange(P, dtype=np.int32)).reshape(P, 1)

        in_maps.append({
            "whT": whT, "wxT": wxT, "bias": bias,
            "carry": carry, "cinit": cinit,
            "gidx": gidx, "ident": np.eye(P, dtype=BF),
            "x0t": x0t,
        })
    return in_maps


def _extract(results):
    """Assemble full output [B, L, D] from stage-3 cores (6, 7)."""
    Y = np.empty((B, L, D), np.float32)
    for j in range(2):
        o = results[6 + j]["out"][NL - 1:NL - 1 + NB]   # [q, p, cols]
        o = o.reshape(NB, P, MT, T, BC).transpose(4, 0, 3, 2, 1)  # [b,q,t,m,p]
        Y[BC * j:BC * (j + 1)] = o.reshape(BC, L, D)
    return Y


def kernel(X, h0s, W, b, _trace=False):
    from concourse.bass_utils import run_bass_kernel_spmd

    if "nc" not in _cache:
        _cache["nc"] = _build()
    nc = _cache["nc"]
    in_maps = _prep_inputs(np.asarray(X), np.asarray(h0s), np.asarray(W),
                           np.asarray(b))
    res = run_bass_kernel_spmd(nc, in_maps, core_ids=list(range(N_CORES)),
                               trace=_trace)
    _cache["last_results"] = res
    return _extract(res.results)
